# revision 38
# baseline (speedup 1.0000x reference)
"""BloomBlock (B=1, S=2048, H=2048, NH=16) on 8 Trainium2 NeuronCores.

Megatron tensor-parallel: each core owns 2 attention heads and 1024 rows of
the 8192-wide MLP. LN1 (+transpose) is replicated; attention/MLP partial
sums are reduce-scattered in bf16; LN2 runs on the local sequence slices;
normalized activations are all-gathered for the MLP.

v2 (throughput rework):
  * All three collectives are CHUNKED over 4 sequence groups of 512 rows
    and issued as soon as their producer chunk finishes, so they overlap
    attention / MLP compute instead of idling every engine. Row ownership
    becomes strided: core c owns global rows {g*512 + c*64 + r}; the host
    reassembles.
  * Alibi and the causal mask are accumulated INTO the score PSUM by the
    tensor engine (a K=2 rank-2 matmul with ones against a bf16
    coarse+fine alibi split, and a transposed-causal-mask matmul), so no
    vector-engine pass over the [128, jw] scores is needed; exp reads
    PSUM directly.
  * Softmax drops the row-max pass: exp uses bias = -alibi[row] (alibi is
    monotonically increasing along the row, scores are O(5), so the
    exponent is bounded above by ~5 and the diagonal term keeps the
    denominator away from 0). The bias is the exact fp32 negation of the
    coarse+fine sum, so the diagonal exponent is exactly 0.
  * LN stats via bn_stats/bn_aggr (one DVE pass instead of reduce_sum +
    Square-accumulate).
  * DMAs consolidated (one descriptor per weight matrix, 2-D/3-D access
    patterns) - the baseline issued 690 DMAs at ~0.7us Sync issue each.
  * PSUM->SBUF copies widened and balanced across Vector/Scalar engines.
  * hidden_states streamed in bf16 (the fp32 residual arrives via res1);
    the v bias is folded into res1 on the host (softmax rows sum to 1).
"""
import sys

for _p in ("/opt/trn_rl_repo",):
    if _p not in sys.path:
        sys.path.insert(0, _p)

import numpy as np
import ml_dtypes

import concourse.bass as bass
from concourse import bacc
import concourse.mybir as mybir
import concourse.tile as tile
from concourse.bass_utils import run_bass_kernel_spmd
from concourse.masks import make_identity, make_lower_triangular

AF = mybir.ActivationFunctionType
ALU = mybir.AluOpType
AX = mybir.AxisListType

B, S, H, NH, HD = 1, 2048, 2048, 16, 128
NCORE = 8
NHC = NH // NCORE          # heads per core = 2
SSH = S // NCORE           # rows owned per core = 256
F1 = 4 * H                 # 8192
F1C = F1 // NCORE          # 1024
NSB = S // 128             # 16 s-blocks
NHCH = H // 128            # 16 h-chunks
NF1 = F1C // 128           # 8
EPS = 1e-5
MASK_NEG = -1e30
G = 4                      # collective chunks (512 rows each)
GROWS = S // G             # 512
CROWS = GROWS // NCORE     # 64 rows per core per chunk

FP = mybir.dt.float32
BF = mybir.dt.bfloat16

SIM_MODE = False           # CoreSim lacks Gelu; use Identity there
PANEL = 512                # stage-A transpose/QKV panel width
NPANEL = S // PANEL        # 4
GORDER = [0, 1, 2, 3]      # ascending: RS1_0/AG_0 fire early, F starts hole-free


def build_program():
    nc = bacc.Bacc("TRN2", target_bir_lowering=False, debug=False,
                   enable_asserts=False, num_devices=NCORE)

    # ---------------- I/O ----------------
    hid = nc.declare_dram_parameter("hid", [S, H], BF, isOutput=False)
    wqk = nc.declare_dram_parameter("wqk", [H, 4 * HD], BF, isOutput=False)
    bqk = nc.declare_dram_parameter("bqk", [4, HD], FP, isOutput=False)
    wv = nc.declare_dram_parameter("wv", [H, NHC * HD], BF, isOutput=False)
    alibsp = nc.declare_dram_parameter("alibsp", [2, NHC * S], BF,
                                       isOutput=False)
    nalib = nc.declare_dram_parameter("nalib", [NHC, NSB, 128], FP,
                                      isOutput=False)
    wd = nc.declare_dram_parameter("wd", [NHC * HD, H], BF, isOutput=False)
    res1 = nc.declare_dram_parameter("res1", [SSH, H], BF, isOutput=False)
    rsel = nc.declare_dram_parameter("rsel", [CROWS, 4 * 128], BF,
                                     isOutput=False)
    wf1 = nc.declare_dram_parameter("wf1", [H, F1C], BF, isOutput=False)
    bf1 = nc.declare_dram_parameter("bf1", [NF1, 128], FP, isOutput=False)
    wf2 = nc.declare_dram_parameter("wf2", [F1C, H], BF, isOutput=False)
    fc2b = nc.declare_dram_parameter("fc2b", [H], FP, isOutput=False)
    out = nc.declare_dram_parameter("out", [SSH, H], FP, isOutput=True)

    rg = [list(range(NCORE))]

    with tile.TileContext(nc) as tc:
        with (
            tc.tile_pool(name="dram", bufs=1, space="DRAM") as dram,
            tc.tile_pool(name="consts", bufs=1) as consts,
            tc.tile_pool(name="stats", bufs=4) as stats,
            tc.tile_pool(name="postp", bufs=1) as postp,
        ):
            # ------- collective bounce buffers (per chunk) -------
            rs1_in = [dram.tile([GROWS, H], BF, tag=f"rs1i{g}", name=f"rs1i{g}")
                      for g in range(G)]
            rs1_out = [dram.tile([CROWS, H], BF, tag=f"rs1o{g}",
                                 name=f"rs1o{g}") for g in range(G)]
            ag_out = [dram.tile([NCORE, CROWS, H], BF, tag=f"ago{g}",
                                name=f"ago{g}", addr_space="Shared")
                      for g in range(G)]
            rs2_in = [dram.tile([GROWS, H], BF, tag=f"rs2i{g}", name=f"rs2i{g}")
                      for g in range(G)]
            rs2_out = [dram.tile([CROWS, H], BF, tag=f"rs2o{g}",
                                 name=f"rs2o{g}") for g in range(G)]

            # ------------ constants (non-DMA first; DMAs after hid) ------
            ident = consts.tile([128, 128], BF, tag="ident")
            make_identity(nc, ident)
            # transposed causal mask: cmT[a,b] = MASK_NEG iff a > b, so
            # (cmT.T @ I)[i,j] = MASK_NEG iff j > i  (strictly-future).
            cmT = consts.tile([128, 128], BF, tag="cmT")
            make_lower_triangular(nc, cmT, val=MASK_NEG, diag=False)
            ones2 = consts.tile([2, 128], BF, tag="ones2")
            nc.vector.memset(ones2, 1.0)
            eps_t = consts.tile([128, 1], FP, tag="eps")
            nc.vector.memset(eps_t, EPS)
            bqk_t = consts.tile([128, 4], FP, tag="bqk")
            nalib_t = consts.tile([128, NHC, NSB], FP, tag="nalib")
            alibsp_t = consts.tile([2, NHC * S], BF, tag="alibsp")
            bf1_t = consts.tile([128, NF1], FP, tag="bf1")
            fc2b_t = consts.tile([128, H], BF, tag="fc2b")

            def load_consts(fstage):
                nc.sync.dma_start(out=bqk_t,
                                  in_=bqk[:, :].rearrange("b p -> p b"))
                nc.sync.dma_start(
                    out=nalib_t,
                    in_=nalib[:, :, :].rearrange("h b p -> p h b"))
                nc.sync.dma_start(out=alibsp_t, in_=alibsp[:, :])
                nc.sync.dma_start(out=bf1_t,
                                  in_=bf1[:, :].rearrange("b p -> p b"))
                nc.gpsimd.dma_start(out=fstage[0:1, :], in_=fc2b[None, :])
                nc.gpsimd.partition_broadcast(fc2b_t, fstage[0:1, :])

            # fp32 attn rows (LN2 input + residual-2), per chunk; spans C..G
            attn_c = [postp.tile([CROWS, H], BF, tag=f"attn{g}",
                                 name=f"attn{g}") for g in range(G)]
            # stage-E scratch; spans C..F (E of the last groups is emitted
            # inside the stage-F stream)
            epool = postp

            def ln_rowstats(src):
                """bn_stats mean/var over the free axis of src [P, H].

                Returns (rstd, -mu*rstd) as [P, 1] fp32."""
                p = src.shape[0]
                bs = stats.tile([128, H // 512, 6], FP, tag="bnst")
                for c in range(H // 512):
                    nc.vector.bn_stats(bs[:p, c, :],
                                       src[:, c * 512:(c + 1) * 512])
                mv = stats.tile([128, 2], FP, tag="bnmv")
                nc.vector.bn_aggr(
                    mv[:p], bs[:p].rearrange("p c s -> p (c s)"))
                std = stats.tile([128, 1], FP, tag="std")
                nc.scalar.activation(std[:p], mv[:p, 1:2], AF.Sqrt,
                                     bias=eps_t[:p])
                rstd = stats.tile([128, 1], FP, tag="rstd")
                nc.vector.reciprocal(rstd[:p], std[:p])
                nmurs = stats.tile([128, 1], FP, tag="nmurs")
                nc.vector.tensor_mul(nmurs[:p], mv[:p, 0:1], rstd[:p])
                nc.vector.tensor_scalar_mul(nmurs[:p], nmurs[:p], -1.0)
                return rstd[:p], nmurs[:p]

            def stage_e(g):
                """AG of the finished (residual-included) attn rows.

                res1 was folded into the dense partial by the selector
                matmul, so rs1_out IS the attention output: the AllGather
                chains directly off the ReduceScatter on the CC stream
                with no intermediate compute."""
                nc.gpsimd.collective_compute(
                    "AllGather", ALU.bypass, replica_groups=rg,
                    ins=[rs1_out[g].opt()], outs=[ag_out[g].opt()])

            # ==== persistent attention activations (stages A..C) ====
            with tc.tile_pool(name="attnp", bufs=1) as attnp:
                qkT = attnp.tile([128, NHC, 2, S], BF, tag="qkT")
                v_t = attnp.tile([128, NSB, NHC * HD], BF, tag="v")
                ctxT = attnp.tile([128, NHC, S], BF, tag="ctxT")
                wd_t = attnp.tile([128, NHC, H], BF, tag="wd")
                r1c_t = attnp.tile([CROWS, G, H], BF, tag="r1c")
                rsel_t = attnp.tile([CROWS, 4, 128], BF, tag="rsel")

                # ==== Stages A+C merged: panel p feeds attention group
                # p immediately (causal: group g needs QKV panels <= g), so
                # RS1_0 fires ~150us earlier and every collective overlaps
                # compute. Stage-A matmuls share the C PSUM rings via tags.
                with (
                    tc.tile_pool(name="workC", bufs=2) as workC,
                    tc.tile_pool(name="psc", bufs=2, space="PSUM") as psc,
                    tc.tile_pool(name="psw", bufs=2, space="PSUM") as psw,
                    tc.tile_pool(name="psx", bufs=2, space="PSUM") as psx,
                ):
                    wpoolA = workA = workC
                    hbs = {}
                    for sb in range(4):  # panel-0 rows first
                        hb = workA.tile([128, H], BF, tag="hidblk", bufs=4,
                                        name=f"hb{sb}")
                        nc.sync.dma_start(
                            out=hb, in_=hid[sb * 128:(sb + 1) * 128, :])
                        hbs[sb] = hb
                    fstage = wpoolA.tile([128, H], BF, tag="fc2bstage", bufs=1)
                    load_consts(fstage)
                    wqk_t = wpoolA.tile([128, NHCH, 4 * HD], BF, tag="wqk", bufs=1)
                    nc.sync.dma_start(
                        out=wqk_t,
                        in_=wqk[:, :].rearrange("(c p) f -> p c f", p=128))
                    wv_t = wpoolA.tile([128, NHCH, NHC * HD], BF, tag="wv", bufs=1)
                    nc.sync.dma_start(
                        out=wv_t,
                        in_=wv[:, :].rearrange("(c p) f -> p c f", p=128))
                    for sb in range(4, NSB):
                        hb = workA.tile([128, H], BF, tag="hidblk", bufs=4,
                                        name=f"hb{sb}")
                        nc.sync.dma_start(
                            out=hb, in_=hid[sb * 128:(sb + 1) * 128, :])
                        hbs[sb] = hb
                    nc.sync.dma_start(
                        out=wd_t,
                        in_=wd[:, :].rearrange("(c p) f -> p c f", p=128))
                    nc.sync.dma_start(
                        out=r1c_t,
                        in_=res1[:, :].rearrange("(g r) h -> r g h", r=CROWS))
                    nc.sync.dma_start(
                        out=rsel_t,
                        in_=rsel[:, :].rearrange("k (b p) -> k b p", p=128))

                    def stage_a_panel(p):
                        xhat_blocks = []
                        for sb4 in range(PANEL // 128):
                            sb = p * (PANEL // 128) + sb4
                            rstd, nmurs = ln_rowstats(hbs[sb])
                            xh = workA.tile([128, H], BF, tag="xhat", bufs=4)
                            nc.scalar.activation(xh, hbs[sb], AF.Identity,
                                                 bias=nmurs, scale=rstd)
                            xhat_blocks.append(xh)

                        xT = workA.tile([128, NHCH, PANEL], BF, tag="xT", bufs=1)
                        for hc in range(NHCH):
                            pt = psw.tile([128, PANEL], BF, tag="wt")
                            for sb4 in range(PANEL // 128):
                                nc.tensor.transpose(
                                    pt[:, sb4 * 128:(sb4 + 1) * 128],
                                    xhat_blocks[sb4][:, hc * 128:(hc + 1) * 128],
                                    ident)
                            if hc % 2 == 0:
                                nc.vector.tensor_copy(xT[:, hc, :], pt)
                            else:
                                nc.scalar.copy(xT[:, hc, :], pt)

                        for fb in range(4):  # q_h0, k_h0, q_h1, k_h1
                            pq = psc.tile([128, PANEL], FP, tag="sc")
                            for hc in range(NHCH):
                                nc.tensor.matmul(
                                    pq, wqk_t[:, hc, fb * 128:(fb + 1) * 128],
                                    xT[:, hc, :],
                                    start=(hc == 0), stop=(hc == NHCH - 1))
                            head, isk = fb // 2, fb % 2
                            nc.scalar.activation(
                                qkT[:, head, isk, p * PANEL:(p + 1) * PANEL],
                                pq, AF.Identity, bias=bqk_t[:, fb:fb + 1],
                                scale=1.0)

                        for sb4 in range(PANEL // 128):
                            blk = p * (PANEL // 128) + sb4
                            pv = psx.tile([128, NHC * HD], FP, tag="acc")
                            for hc in range(NHCH):
                                nc.tensor.matmul(
                                    pv, xT[:, hc, sb4 * 128:(sb4 + 1) * 128],
                                    wv_t[:, hc, :],
                                    start=(hc == 0), stop=(hc == NHCH - 1))
                            nc.vector.tensor_copy(v_t[:, blk, :], pv)

                    # ==== attention + dense + chunked RS1/AG ====
                    def attention_head(g, head):
                        """scores/softmax/ctx for q-rows [g*512,(g+1)*512)."""
                        probs_g, dn_g = [], []
                        for ib in range(4 * g, 4 * g + 4):
                            jw = (ib + 1) * 128
                            qblk = qkT[:, head, 0, ib * 128:(ib + 1) * 128]
                            nch = (jw + 1023) // 1024
                            pch = []
                            for cc in range(nch):
                                c0 = cc * 1024
                                w_ = min(1024, jw - c0)
                                pp = psc.tile([128, 1024], FP, tag="sc")
                                pch.append((pp, c0, w_))
                            # pass 1: scores (stationary q-block)
                            for pp, c0, w_ in pch:
                                for s0 in range(0, w_, 512):
                                    sw = min(512, w_ - s0)
                                    nc.tensor.matmul(
                                        pp[:, s0:s0 + sw], qblk,
                                        qkT[:, head, 1, c0 + s0:c0 + s0 + sw],
                                        start=True, stop=False,
                                        skip_group_check=True)
                            # pass 2: + alibi (rank-2: coarse+fine rows)
                            for pp, c0, w_ in pch:
                                for s0 in range(0, w_, 512):
                                    sw = min(512, w_ - s0)
                                    j0 = head * S + c0 + s0
                                    isdiag = (c0 + s0 + sw == jw)
                                    nc.tensor.matmul(
                                        pp[:, s0:s0 + sw], ones2,
                                        alibsp_t[:, j0:j0 + sw],
                                        start=False, stop=not isdiag,
                                        skip_group_check=True)
                            # pass 3: causal mask on the diagonal block
                            pp, c0, w_ = pch[-1]
                            d0 = (jw - 128) - c0
                            nc.tensor.matmul(
                                pp[:, d0:d0 + 128], cmT, ident,
                                start=False, stop=True,
                                skip_group_check=True)
                            # exp (PSUM-read) + denominators
                            probs = workC.tile([128, S], BF, tag="probs",
                                               bufs=4)
                            dparts = []
                            for ci, (pp, c0, w_) in enumerate(pch):
                                dp = stats.tile([128, 1], FP, tag=f"dnm{ci}",
                                                name=f"dnm{ci}")
                                nc.scalar.activation(
                                    probs[:, c0:c0 + w_], pp[:, :w_], AF.Exp,
                                    bias=nalib_t[:, head, ib:ib + 1],
                                    scale=1.0, accum_out=dp)
                                dparts.append(dp)
                            if len(dparts) == 2:
                                nc.vector.tensor_add(dparts[0], dparts[0],
                                                     dparts[1])
                            invd = stats.tile([128, 1], FP, tag="invd")
                            nc.vector.reciprocal(invd, dparts[0])
                            dn = workC.tile([128, 128], BF, tag="dn", bufs=5)
                            nc.vector.tensor_scalar_mul(dn, ident, invd)
                            probs_g.append(probs)
                            dn_g.append(dn)

                        pctx = psx.tile([128, 512], FP, tag="acc")
                        njc = 4 * g + 4
                        for jc in range(njc):
                            ib0 = max(jc, 4 * g)
                            nblk = 4 * g + 4 - ib0
                            pw = psw.tile([128, 512], FP, tag="wt")
                            for k, ib in enumerate(range(ib0, 4 * g + 4)):
                                nc.tensor.matmul(
                                    pw[:, k * 128:(k + 1) * 128],
                                    probs_g[ib - 4 * g][:, jc * 128:(jc + 1) * 128],
                                    dn_g[ib - 4 * g],
                                    start=True, stop=True)
                            wts = workC.tile([128, 512], BF, tag="wts", bufs=3)
                            nc.vector.tensor_copy(wts[:, :nblk * 128],
                                                  pw[:, :nblk * 128])
                            off = (ib0 - 4 * g) * 128
                            nc.tensor.matmul(
                                pctx[:, off:off + nblk * 128],
                                v_t[:, jc, head * HD:(head + 1) * HD],
                                wts[:, :nblk * 128],
                                start=(jc == 0), stop=(jc == njc - 1),
                                skip_group_check=True)
                        nc.vector.tensor_copy(
                            ctxT[:, head, g * 512:(g + 1) * 512], pctx)

                    def dense_rs1(g):
                        """dense partial for rows [g*512,(g+1)*512) + RS1.

                        A final K=64 one-hot selector matmul adds res1 for
                        exactly the rows this core owns, so rs1_out is the
                        FINISHED attention row and the AllGather can chain
                        directly off the ReduceScatter."""
                        dsb = workC.tile([128, 4, H], BF, tag="densebf", bufs=1)
                        for sb4 in range(4):
                            blk = 4 * g + sb4
                            for fc in range(4):
                                pd = psx.tile([128, 512], FP, tag="acc")
                                for h in range(NHC):
                                    nc.tensor.matmul(
                                        pd,
                                        ctxT[:, h, blk * 128:(blk + 1) * 128],
                                        wd_t[:, h, fc * 512:(fc + 1) * 512],
                                        start=(h == 0), stop=False,
                                        skip_group_check=True)
                                nc.tensor.matmul(
                                    pd, rsel_t[:, sb4, :],
                                    r1c_t[:, g, fc * 512:(fc + 1) * 512],
                                    start=False, stop=True,
                                    skip_group_check=True)
                                if fc % 2 == 0:
                                    nc.vector.tensor_copy(
                                        dsb[:, sb4, fc * 512:(fc + 1) * 512],
                                        pd)
                                else:
                                    nc.scalar.copy(
                                        dsb[:, sb4, fc * 512:(fc + 1) * 512],
                                        pd)
                        nc.sync.dma_start(
                            out=rs1_in[g][:, :].rearrange("(b p) h -> p b h",
                                                          p=128),
                            in_=dsb)
                        nc.gpsimd.collective_compute(
                            "ReduceScatter", ALU.add, replica_groups=rg,
                            ins=[rs1_in[g].opt()], outs=[rs1_out[g].opt()])

                    # stage_e is pure-gpsimd, so it can follow its dense
                    # immediately: AG_g fires at the earliest possible
                    # moment and no compute queue ever blocks on it.
                    for idx, g in enumerate(GORDER):
                        stage_a_panel(g)
                        for head in range(NHC):
                            attention_head(g, head)
                        dense_rs1(g)
                        stage_e(g)

            # ======== Stage F: MLP + chunked RS2; Stage G epilogue ========
            with (
                tc.tile_pool(name="mlpw", bufs=1) as mlpw,
                tc.tile_pool(name="workF", bufs=2) as workF,
                tc.tile_pool(name="pf", bufs=2, space="PSUM") as pf_pool,
                tc.tile_pool(name="pt3", bufs=2, space="PSUM") as pt3_pool,
                tc.tile_pool(name="pm", bufs=2, space="PSUM") as pm_pool,
            ):
                # first chunk's gather BEFORE the weights: it unblocks the
                # transposes; wf1's first half follows (FC1 needs it next),
                # then wf2 (needed ~27us later).
                g0 = GORDER[0]
                ytmp0 = workF.tile([128, 4, H], BF, tag="ytmp", bufs=1,
                                   name="ytmp0")
                nc.sync.dma_start(
                    out=ytmp0,
                    in_=ag_out[g0][:, :, :].rearrange(
                        "c r h -> (c r) h").rearrange(
                        "(b p) h -> p b h", p=128))
                wf1_t = mlpw.tile([128, NHCH, F1C], BF, tag="wf1")
                nc.sync.dma_start(
                    out=wf1_t[:, :, :F1C // 2],
                    in_=wf1[:, :F1C // 2].rearrange("(c p) f -> p c f", p=128))
                nc.sync.dma_start(
                    out=wf1_t[:, :, F1C // 2:],
                    in_=wf1[:, F1C // 2:].rearrange("(c p) f -> p c f", p=128))
                wf2_t = mlpw.tile([128, NF1, H], BF, tag="wf2")
                nc.sync.dma_start(
                    out=wf2_t,
                    in_=wf2[:, :].rearrange("(c p) f -> p c f", p=128))

                def stage_g(g):
                    """rows of chunk g: out = rs2 + attn(+fc2b).

                    rs2 read waits on the RS2_g collective -> gpsimd queue."""
                    rsb2 = workF.tile([CROWS, H], BF, tag="rs2blk")
                    nc.gpsimd.dma_start(out=rsb2, in_=rs2_out[g])
                    ob = workF.tile([CROWS, H], FP, tag="outblk", bufs=1)
                    nc.gpsimd.tensor_add(ob, rsb2, attn_c[g])
                    nc.gpsimd.dma_start(out=out[g * CROWS:(g + 1) * CROWS, :],
                                        in_=ob)

                def prep(g, ytmp_pre=None):
                    """gather + replicated LN2 + transpose -> yTs for chunk g.

                    Emitted one chunk ahead (between FC1 and FC2 of the
                    previous chunk) so the DVE/ACT LN work hides under the
                    previous chunk's matmuls."""
                    if ytmp_pre is not None:
                        ytmp = ytmp_pre
                    else:
                        ytmp = workF.tile([128, 4, H], BF, tag="ytmp", bufs=1)
                        nc.gpsimd.dma_start(
                            out=ytmp,
                            in_=ag_out[g][:, :, :].rearrange(
                                "c r h -> (c r) h").rearrange(
                                "(b p) h -> p b h", p=128))
                    for b4 in range(4):
                        rstd, nmurs = ln_rowstats(ytmp[:, b4, :])
                        nc.scalar.activation(ytmp[:, b4, :], ytmp[:, b4, :],
                                             AF.Identity, bias=nmurs,
                                             scale=rstd)
                    yTs = workF.tile([128, NHCH, 512], BF, tag="yTs", bufs=2)
                    for hc in range(NHCH):
                        pt3 = pt3_pool.tile([128, 512], BF, tag="mmT3")
                        for b4 in range(4):
                            nc.tensor.transpose(
                                pt3[:, b4 * 128:(b4 + 1) * 128],
                                ytmp[:, b4, hc * 128:(hc + 1) * 128],
                                ident)
                        if hc % 2 == 0:
                            nc.vector.tensor_copy(yTs[:, hc, :], pt3)
                        else:
                            nc.scalar.copy(yTs[:, hc, :], pt3)
                    return yTs

                yTs_cur = prep(GORDER[0], ytmp_pre=ytmp0)
                for idx, g in enumerate(GORDER):
                    # fetch this core's finished attn rows (residual-2) and
                    # fold fc2b in (overlapped, off the tail)
                    nc.gpsimd.dma_start(out=attn_c[g], in_=rs1_out[g])
                    nc.gpsimd.tensor_add(attn_c[g], attn_c[g],
                                         fc2b_t[0:CROWS, :])
                    # FC1: hdnT[f1, seq-chunk] = gelu(Wf1 @ yT + b)
                    hdnT = workF.tile([128, NF1, 512], BF, tag="hdnT")
                    for f1c in range(NF1):
                        pf = pf_pool.tile([128, 512], FP, tag="mmf1")
                        for hc in range(NHCH):
                            nc.tensor.matmul(
                                pf, wf1_t[:, hc, f1c * 128:(f1c + 1) * 128],
                                yTs_cur[:, hc, :],
                                start=(hc == 0), stop=(hc == NHCH - 1))
                        nc.scalar.activation(
                            hdnT[:, f1c, :], pf,
                            AF.Identity if SIM_MODE else AF.Gelu_apprx_tanh,
                            bias=bf1_t[:, f1c:f1c + 1], scale=1.0)
                    # FC2 partial + RS2_g
                    for sb4 in range(4):
                        msb = workF.tile([128, H], BF, tag="mlpbf", bufs=2)
                        for half in range(2):
                            pm = pm_pool.tile([128, 1024], FP, tag="mmf2")
                            for f1c in range(NF1):
                                st = hdnT[:, f1c, sb4 * 128:(sb4 + 1) * 128]
                                for fc in range(2):
                                    f0 = half * 1024 + fc * 512
                                    nc.tensor.matmul(
                                        pm[:, fc * 512:(fc + 1) * 512],
                                        st, wf2_t[:, f1c, f0:f0 + 512],
                                        start=(f1c == 0),
                                        stop=(f1c == NF1 - 1),
                                        skip_group_check=True)
                            if half == 0:
                                nc.vector.tensor_copy(
                                    msb[:, half * 1024:(half + 1) * 1024], pm)
                            else:
                                nc.scalar.copy(
                                    msb[:, half * 1024:(half + 1) * 1024], pm)
                        nc.sync.dma_start(
                            out=rs2_in[g][sb4 * 128:(sb4 + 1) * 128, :],
                            in_=msb)
                    nc.gpsimd.collective_compute(
                        "ReduceScatter", ALU.add, replica_groups=rg,
                        ins=[rs2_in[g].opt()], outs=[rs2_out[g].opt()])
                    if idx >= 1:
                        stage_g(GORDER[idx - 1])
                    # next chunk's gather/LN2/transpose: AFTER this chunk's
                    # matmuls in the PE stream, so a late AllGather can only
                    # stall work that genuinely needs it.
                    if idx + 1 < len(GORDER):
                        yTs_cur = prep(GORDER[idx + 1])
                stage_g(GORDER[-1])
    nc.compile()
    return nc


def _host_prep(inputs):
    """Slice/fold weights per core. Returns list of per-core input maps."""
    bf16 = ml_dtypes.bfloat16
    hs = np.asarray(inputs["hidden_states"], np.float32).reshape(S, H)
    g1 = np.asarray(inputs["ln1_g"], np.float32)
    b1 = np.asarray(inputs["ln1_b"], np.float32)
    qkv_w = np.asarray(inputs["qkv_w"], np.float32)
    qkv_b = np.asarray(inputs["qkv_b"], np.float32)
    dense_w = np.asarray(inputs["dense_w"], np.float32)
    dense_b = np.asarray(inputs["dense_b"], np.float32)
    g2 = np.asarray(inputs["ln2_g"], np.float32)
    b2 = np.asarray(inputs["ln2_b"], np.float32)
    fc1_w = np.asarray(inputs["fc1_w"], np.float32)
    fc1_b = np.asarray(inputs["fc1_b"], np.float32)
    fc2_w = np.asarray(inputs["fc2_w"], np.float32)
    fc2_b = np.asarray(inputs["fc2_b"], np.float32)
    alibi = np.asarray(inputs["alibi"], np.float32).reshape(NH, S)

    inv = 1.0 / np.sqrt(np.float32(HD))
    hs_bf = hs.astype(bf16)

    # v-bias contribution to the dense output (softmax rows sum to 1):
    # ctx = ctx_nobv + bv  =>  dense += bv_cat @ dense_w.T  (fold into res1)
    bv_cat = np.zeros(H, np.float32)
    for h in range(NH):
        vr = qkv_w[h * 3 * HD + 2 * HD:h * 3 * HD + 3 * HD, :]
        bv_cat[h * HD:(h + 1) * HD] = (
            qkv_b[h * 3 * HD + 2 * HD:h * 3 * HD + 3 * HD] + vr @ b1)
    dense_b_eff = dense_b + bv_cat @ dense_w.T

    in_maps = []
    for c in range(NCORE):
        heads = [NHC * c + i for i in range(NHC)]
        wqk_cols, bqk_rows, wv_cols = [], [], []
        for h in heads:
            qr = qkv_w[h * 3 * HD:h * 3 * HD + HD, :]
            kr = qkv_w[h * 3 * HD + HD:h * 3 * HD + 2 * HD, :]
            vr = qkv_w[h * 3 * HD + 2 * HD:h * 3 * HD + 3 * HD, :]
            qb = qkv_b[h * 3 * HD:h * 3 * HD + HD] + qr @ b1
            kb = qkv_b[h * 3 * HD + HD:h * 3 * HD + 2 * HD] + kr @ b1
            wqk_cols.append((qr * g1[None, :]).T * inv)
            wqk_cols.append((kr * g1[None, :]).T)
            bqk_rows.append(qb * inv)
            bqk_rows.append(kb)
            wv_cols.append((vr * g1[None, :]).T)
        rows_c = np.concatenate(
            [np.arange(g * GROWS + c * CROWS, g * GROWS + (c + 1) * CROWS)
             for g in range(G)])
        # one-hot selector: within a 512-row chunk, this core owns rows
        # [c*64, (c+1)*64) -> block b4=c//2, partitions (c%2)*64 + k
        rsel_np = np.zeros((CROWS, 4, 128), np.float32)
        for k in range(CROWS):
            rsel_np[k, c // 2, (c % 2) * CROWS + k] = 1.0
        rsel_c = np.ascontiguousarray(rsel_np.reshape(CROWS, 512)).astype(bf16)
        alibi_c = alibi[heads[0]:heads[-1] + 1, :]          # [NHC, S]
        # coarse part: multiples of 8 (exact in bf16 up to 2048);
        # fine part in [-4, 4] (bf16 rounding ~2^-8 relative).
        acoarse = (8.0 * np.round(alibi_c / 8.0)).astype(bf16)
        afine = (alibi_c - acoarse.astype(np.float32)).astype(bf16)
        # the exp bias is the exact fp32 negation of the on-device sum
        nalib_c = -(acoarse.astype(np.float32) + afine.astype(np.float32))
        in_maps.append({
            "hid": hs_bf,
            "wqk": np.ascontiguousarray(
                np.concatenate(wqk_cols, axis=1)).astype(bf16),
            "bqk": np.ascontiguousarray(np.stack(bqk_rows, axis=0)),
            "wv": np.ascontiguousarray(
                np.concatenate(wv_cols, axis=1)).astype(bf16),
            "alibsp": np.ascontiguousarray(
                np.stack([acoarse.reshape(-1), afine.reshape(-1)], axis=0)),
            "nalib": np.ascontiguousarray(nalib_c.reshape(NHC, NSB, 128)),
            "wd": np.ascontiguousarray(
                dense_w[:, heads[0] * HD:(heads[-1] + 1) * HD].T).astype(bf16),
            "res1": np.ascontiguousarray(
                hs[rows_c, :] + dense_b_eff[None, :]).astype(bf16),
            "rsel": rsel_c,
            "wf1": np.ascontiguousarray(
                (fc1_w[c * F1C:(c + 1) * F1C, :] * g2[None, :]).T).astype(bf16),
            "bf1": np.ascontiguousarray(
                (fc1_b[c * F1C:(c + 1) * F1C]
                 + fc1_w[c * F1C:(c + 1) * F1C, :] @ b2
                 ).reshape(NF1, 128)),
            "wf2": np.ascontiguousarray(
                fc2_w[:, c * F1C:(c + 1) * F1C].T).astype(bf16),
            "fc2b": fc2_b,
        })
    return in_maps


def _assemble(results) -> np.ndarray:
    """Reassemble the strided row ownership into the full [B, S, H]."""
    full = np.empty((S, H), np.float32)
    for c in range(NCORE):
        shard = np.asarray(results[c]["out"], np.float32)  # [SSH, H]
        for g in range(G):
            full[g * GROWS + c * CROWS:g * GROWS + (c + 1) * CROWS, :] = \
                shard[g * CROWS:(g + 1) * CROWS, :]
    return full.reshape(B, S, H)


_CACHED_NC = None


def kernel(**inputs) -> np.ndarray:
    global _CACHED_NC
    in_maps = _host_prep(inputs)
    if _CACHED_NC is None:
        _CACHED_NC = build_program()
    res = run_bass_kernel_spmd(_CACHED_NC, in_maps, list(range(NCORE)))
    return _assemble(res.results)


# revision 39
# speedup vs baseline: 1.0195x; 1.0195x over previous
"""BloomBlock (B=1, S=2048, H=2048, NH=16) on 8 Trainium2 NeuronCores.

Megatron tensor-parallel: each core owns 2 attention heads and 1024 rows of
the 8192-wide MLP. LN1 (+transpose) is replicated; attention/MLP partial
sums are reduce-scattered in bf16; LN2 runs on the local sequence slices;
normalized activations are all-gathered for the MLP.

v2 (throughput rework):
  * All three collectives are CHUNKED over 4 sequence groups of 512 rows
    and issued as soon as their producer chunk finishes, so they overlap
    attention / MLP compute instead of idling every engine. Row ownership
    becomes strided: core c owns global rows {g*512 + c*64 + r}; the host
    reassembles.
  * Alibi and the causal mask are accumulated INTO the score PSUM by the
    tensor engine (a K=2 rank-2 matmul with ones against a bf16
    coarse+fine alibi split, and a transposed-causal-mask matmul), so no
    vector-engine pass over the [128, jw] scores is needed; exp reads
    PSUM directly.
  * Softmax drops the row-max pass: exp uses bias = -alibi[row] (alibi is
    monotonically increasing along the row, scores are O(5), so the
    exponent is bounded above by ~5 and the diagonal term keeps the
    denominator away from 0). The bias is the exact fp32 negation of the
    coarse+fine sum, so the diagonal exponent is exactly 0.
  * LN stats via bn_stats/bn_aggr (one DVE pass instead of reduce_sum +
    Square-accumulate).
  * DMAs consolidated (one descriptor per weight matrix, 2-D/3-D access
    patterns) - the baseline issued 690 DMAs at ~0.7us Sync issue each.
  * PSUM->SBUF copies widened and balanced across Vector/Scalar engines.
  * hidden_states streamed in bf16 (the fp32 residual arrives via res1);
    the v bias is folded into res1 on the host (softmax rows sum to 1).
"""
import sys

for _p in ("/opt/trn_rl_repo",):
    if _p not in sys.path:
        sys.path.insert(0, _p)

import numpy as np
import ml_dtypes

import concourse.bass as bass
from concourse import bacc
import concourse.mybir as mybir
import concourse.tile as tile
from concourse.bass_utils import run_bass_kernel_spmd
from concourse.masks import make_identity, make_lower_triangular

AF = mybir.ActivationFunctionType
ALU = mybir.AluOpType
AX = mybir.AxisListType

B, S, H, NH, HD = 1, 2048, 2048, 16, 128
NCORE = 8
NHC = NH // NCORE          # heads per core = 2
SSH = S // NCORE           # rows owned per core = 256
F1 = 4 * H                 # 8192
F1C = F1 // NCORE          # 1024
NSB = S // 128             # 16 s-blocks
NHCH = H // 128            # 16 h-chunks
NF1 = F1C // 128           # 8
EPS = 1e-5
MASK_NEG = -1e30
G = 4                      # collective chunks (512 rows each)
GROWS = S // G             # 512
CROWS = GROWS // NCORE     # 64 rows per core per chunk

FP = mybir.dt.float32
BF = mybir.dt.bfloat16

SIM_MODE = False           # CoreSim lacks Gelu; use Identity there
PANEL = 512                # stage-A transpose/QKV panel width
NPANEL = S // PANEL        # 4
GORDER = [0, 1, 2, 3]      # ascending: RS1_0/AG_0 fire early, F starts hole-free


def build_program():
    nc = bacc.Bacc("TRN2", target_bir_lowering=False, debug=False,
                   enable_asserts=False, num_devices=NCORE)

    # ---------------- I/O ----------------
    hid = nc.declare_dram_parameter("hid", [S, H], BF, isOutput=False)
    wqk = nc.declare_dram_parameter("wqk", [H, 4 * HD], BF, isOutput=False)
    bqk = nc.declare_dram_parameter("bqk", [4, HD], FP, isOutput=False)
    wv = nc.declare_dram_parameter("wv", [H, NHC * HD], BF, isOutput=False)
    alibsp = nc.declare_dram_parameter("alibsp", [2, NHC * S], BF,
                                       isOutput=False)
    nalib = nc.declare_dram_parameter("nalib", [NHC, NSB, 128], FP,
                                      isOutput=False)
    wd = nc.declare_dram_parameter("wd", [NHC * HD, H], BF, isOutput=False)
    res1 = nc.declare_dram_parameter("res1", [SSH, H], BF, isOutput=False)
    rsel = nc.declare_dram_parameter("rsel", [CROWS, 4 * 128], BF,
                                     isOutput=False)
    wf1 = nc.declare_dram_parameter("wf1", [H, F1C], BF, isOutput=False)
    bf1 = nc.declare_dram_parameter("bf1", [NF1, 128], FP, isOutput=False)
    wf2 = nc.declare_dram_parameter("wf2", [F1C, H], BF, isOutput=False)
    fc2b = nc.declare_dram_parameter("fc2b", [H], FP, isOutput=False)
    out = nc.declare_dram_parameter("out", [SSH, H], FP, isOutput=True)

    rg = [list(range(NCORE))]

    with tile.TileContext(nc) as tc:
        with (
            tc.tile_pool(name="dram", bufs=1, space="DRAM") as dram,
            tc.tile_pool(name="consts", bufs=1) as consts,
            tc.tile_pool(name="stats", bufs=4) as stats,
            tc.tile_pool(name="postp", bufs=1) as postp,
        ):
            # ------- collective bounce buffers (per chunk) -------
            rs1_in = [dram.tile([GROWS, H], BF, tag=f"rs1i{g}", name=f"rs1i{g}")
                      for g in range(G)]
            rs1_out = [dram.tile([CROWS, H], BF, tag=f"rs1o{g}",
                                 name=f"rs1o{g}") for g in range(G)]
            ag_out = [dram.tile([NCORE, CROWS, H], BF, tag=f"ago{g}",
                                name=f"ago{g}", addr_space="Shared")
                      for g in range(G)]
            rs2_in = [dram.tile([GROWS, H], BF, tag=f"rs2i{g}", name=f"rs2i{g}")
                      for g in range(G)]
            rs2_out = [dram.tile([CROWS, H], BF, tag=f"rs2o{g}",
                                 name=f"rs2o{g}") for g in range(G)]

            # ------------ constants (non-DMA first; DMAs after hid) ------
            ident = consts.tile([128, 128], BF, tag="ident")
            make_identity(nc, ident)
            # transposed causal mask: cmT[a,b] = MASK_NEG iff a > b, so
            # (cmT.T @ I)[i,j] = MASK_NEG iff j > i  (strictly-future).
            cmT = consts.tile([128, 128], BF, tag="cmT")
            make_lower_triangular(nc, cmT, val=MASK_NEG, diag=False)
            ones2 = consts.tile([2, 128], BF, tag="ones2")
            nc.vector.memset(ones2, 1.0)
            eps_t = consts.tile([128, 1], FP, tag="eps")
            nc.vector.memset(eps_t, EPS)
            bqk_t = consts.tile([128, 4], FP, tag="bqk")
            nalib_t = consts.tile([128, NHC, NSB], FP, tag="nalib")
            alibsp_t = consts.tile([2, NHC * S], BF, tag="alibsp")
            bf1_t = consts.tile([128, NF1], FP, tag="bf1")
            fc2b_t = consts.tile([128, H], FP, tag="fc2b")

            def load_consts(fstage):
                nc.sync.dma_start(out=bqk_t,
                                  in_=bqk[:, :].rearrange("b p -> p b"))
                nc.sync.dma_start(
                    out=nalib_t,
                    in_=nalib[:, :, :].rearrange("h b p -> p h b"))
                nc.sync.dma_start(out=alibsp_t, in_=alibsp[:, :])
                nc.sync.dma_start(out=bf1_t,
                                  in_=bf1[:, :].rearrange("b p -> p b"))
                nc.sync.dma_start(out=fstage[0:1, :], in_=fc2b[None, :])
                nc.gpsimd.partition_broadcast(fc2b_t, fstage[0:1, :])

            # fp32 attn rows (LN2 input + residual-2), per chunk; spans C..G
            attn_c = [postp.tile([CROWS, H], BF, tag=f"attn{g}",
                                 name=f"attn{g}") for g in range(G)]
            # stage-E scratch; spans C..F (E of the last groups is emitted
            # inside the stage-F stream)
            epool = postp

            def ln_rowstats(src):
                """bn_stats mean/var over the free axis of src [P, H].

                Returns (rstd, -mu*rstd) as [P, 1] fp32."""
                p = src.shape[0]
                bs = stats.tile([128, H // 512, 6], FP, tag="bnst")
                for c in range(H // 512):
                    nc.vector.bn_stats(bs[:p, c, :],
                                       src[:, c * 512:(c + 1) * 512])
                mv = stats.tile([128, 2], FP, tag="bnmv")
                nc.vector.bn_aggr(
                    mv[:p], bs[:p].rearrange("p c s -> p (c s)"))
                std = stats.tile([128, 1], FP, tag="std")
                nc.scalar.activation(std[:p], mv[:p, 1:2], AF.Sqrt,
                                     bias=eps_t[:p])
                rstd = stats.tile([128, 1], FP, tag="rstd")
                nc.vector.reciprocal(rstd[:p], std[:p])
                nmurs = stats.tile([128, 1], FP, tag="nmurs")
                nc.vector.tensor_mul(nmurs[:p], mv[:p, 0:1], rstd[:p])
                nc.vector.tensor_scalar_mul(nmurs[:p], nmurs[:p], -1.0)
                return rstd[:p], nmurs[:p]

            def stage_e(g):
                """AG of the finished (residual-included) attn rows.

                res1 was folded into the dense partial by the selector
                matmul, so rs1_out IS the attention output: the AllGather
                chains directly off the ReduceScatter on the CC stream
                with no intermediate compute."""
                nc.gpsimd.collective_compute(
                    "AllGather", ALU.bypass, replica_groups=rg,
                    ins=[rs1_out[g].opt()], outs=[ag_out[g].opt()])

            # ==== persistent attention activations (stages A..C) ====
            with tc.tile_pool(name="attnp", bufs=1) as attnp:
                qkT = attnp.tile([128, NHC, 2, S], BF, tag="qkT")
                v_t = attnp.tile([128, NSB, NHC * HD], BF, tag="v")
                ctxT = attnp.tile([128, NHC, S], BF, tag="ctxT")
                wd_t = attnp.tile([128, NHC, H], BF, tag="wd")
                r1c_t = attnp.tile([CROWS, G, H], BF, tag="r1c")
                rsel_t = attnp.tile([CROWS, 4, 128], BF, tag="rsel")

                # ---- Stage A: hid DMA, LN1, transpose, QKV per panel ----
                with (
                    tc.tile_pool(name="wpoolA", bufs=1) as wpoolA,
                    tc.tile_pool(name="workA", bufs=2) as workA,
                    tc.tile_pool(name="pA_t", bufs=2, space="PSUM") as pA_t,
                    tc.tile_pool(name="pA_qk", bufs=2, space="PSUM") as pA_qk,
                    tc.tile_pool(name="pA_v", bufs=2, space="PSUM") as pA_v,
                ):
                    hbs = {}
                    for sb in range(4):  # panel-0 rows first
                        hb = workA.tile([128, H], BF, tag="hidblk", bufs=4,
                                        name=f"hb{sb}")
                        nc.sync.dma_start(
                            out=hb, in_=hid[sb * 128:(sb + 1) * 128, :])
                        hbs[sb] = hb
                    fstage = wpoolA.tile([128, H], FP, tag="fc2bstage")
                    load_consts(fstage)
                    wqk_t = wpoolA.tile([128, NHCH, 4 * HD], BF, tag="wqk")
                    nc.sync.dma_start(
                        out=wqk_t,
                        in_=wqk[:, :].rearrange("(c p) f -> p c f", p=128))
                    wv_t = wpoolA.tile([128, NHCH, NHC * HD], BF, tag="wv")
                    nc.sync.dma_start(
                        out=wv_t,
                        in_=wv[:, :].rearrange("(c p) f -> p c f", p=128))
                    for sb in range(4, NSB):
                        hb = workA.tile([128, H], BF, tag="hidblk", bufs=4,
                                        name=f"hb{sb}")
                        nc.sync.dma_start(
                            out=hb, in_=hid[sb * 128:(sb + 1) * 128, :])
                        hbs[sb] = hb
                    nc.sync.dma_start(
                        out=wd_t,
                        in_=wd[:, :].rearrange("(c p) f -> p c f", p=128))
                    nc.sync.dma_start(
                        out=r1c_t,
                        in_=res1[:, :].rearrange("(g r) h -> r g h", r=CROWS))
                    nc.sync.dma_start(
                        out=rsel_t,
                        in_=rsel[:, :].rearrange("k (b p) -> k b p", p=128))

                    for p in range(NPANEL):
                        xhat_blocks = []
                        for sb4 in range(PANEL // 128):
                            sb = p * (PANEL // 128) + sb4
                            rstd, nmurs = ln_rowstats(hbs[sb])
                            xh = workA.tile([128, H], BF, tag="xhat", bufs=4)
                            nc.scalar.activation(xh, hbs[sb], AF.Identity,
                                                 bias=nmurs, scale=rstd)
                            xhat_blocks.append(xh)

                        xT = workA.tile([128, NHCH, PANEL], BF, tag="xT")
                        for hc in range(NHCH):
                            pt = pA_t.tile([128, PANEL], BF, tag="mmT")
                            for sb4 in range(PANEL // 128):
                                nc.tensor.transpose(
                                    pt[:, sb4 * 128:(sb4 + 1) * 128],
                                    xhat_blocks[sb4][:, hc * 128:(hc + 1) * 128],
                                    ident)
                            if hc % 2 == 0:
                                nc.vector.tensor_copy(xT[:, hc, :], pt)
                            else:
                                nc.scalar.copy(xT[:, hc, :], pt)

                        for fb in range(4):  # q_h0, k_h0, q_h1, k_h1
                            pq = pA_qk.tile([128, PANEL], FP, tag="mmqk")
                            for hc in range(NHCH):
                                nc.tensor.matmul(
                                    pq, wqk_t[:, hc, fb * 128:(fb + 1) * 128],
                                    xT[:, hc, :],
                                    start=(hc == 0), stop=(hc == NHCH - 1))
                            head, isk = fb // 2, fb % 2
                            nc.scalar.activation(
                                qkT[:, head, isk, p * PANEL:(p + 1) * PANEL],
                                pq, AF.Identity, bias=bqk_t[:, fb:fb + 1],
                                scale=1.0)

                        for sb4 in range(PANEL // 128):
                            blk = p * (PANEL // 128) + sb4
                            pv = pA_v.tile([128, NHC * HD], FP, tag="mmv")
                            for hc in range(NHCH):
                                nc.tensor.matmul(
                                    pv, xT[:, hc, sb4 * 128:(sb4 + 1) * 128],
                                    wv_t[:, hc, :],
                                    start=(hc == 0), stop=(hc == NHCH - 1))
                            nc.vector.tensor_copy(v_t[:, blk, :], pv)

                # ==== Stage C/D/E: attention + dense + chunked RS1/AG ====
                with (
                    tc.tile_pool(name="workC", bufs=2) as workC,
                    tc.tile_pool(name="psc", bufs=2, space="PSUM") as psc,
                    tc.tile_pool(name="psw", bufs=2, space="PSUM") as psw,
                    tc.tile_pool(name="psx", bufs=2, space="PSUM") as psx,
                ):
                    def attention_head(g, head):
                        """scores/softmax/ctx for q-rows [g*512,(g+1)*512)."""
                        probs_g, dn_g = [], []
                        for ib in range(4 * g, 4 * g + 4):
                            jw = (ib + 1) * 128
                            qblk = qkT[:, head, 0, ib * 128:(ib + 1) * 128]
                            nch = (jw + 1023) // 1024
                            pch = []
                            for cc in range(nch):
                                c0 = cc * 1024
                                w_ = min(1024, jw - c0)
                                pp = psc.tile([128, 1024], FP, tag="sc")
                                pch.append((pp, c0, w_))
                            # pass 1: scores (stationary q-block)
                            for pp, c0, w_ in pch:
                                for s0 in range(0, w_, 512):
                                    sw = min(512, w_ - s0)
                                    nc.tensor.matmul(
                                        pp[:, s0:s0 + sw], qblk,
                                        qkT[:, head, 1, c0 + s0:c0 + s0 + sw],
                                        start=True, stop=False,
                                        skip_group_check=True)
                            # pass 2: + alibi (rank-2: coarse+fine rows)
                            for pp, c0, w_ in pch:
                                for s0 in range(0, w_, 512):
                                    sw = min(512, w_ - s0)
                                    j0 = head * S + c0 + s0
                                    isdiag = (c0 + s0 + sw == jw)
                                    nc.tensor.matmul(
                                        pp[:, s0:s0 + sw], ones2,
                                        alibsp_t[:, j0:j0 + sw],
                                        start=False, stop=not isdiag,
                                        skip_group_check=True)
                            # pass 3: causal mask on the diagonal block
                            pp, c0, w_ = pch[-1]
                            d0 = (jw - 128) - c0
                            nc.tensor.matmul(
                                pp[:, d0:d0 + 128], cmT, ident,
                                start=False, stop=True,
                                skip_group_check=True)
                            # exp (PSUM-read) + denominators
                            probs = workC.tile([128, S], BF, tag="probs",
                                               bufs=4)
                            dparts = []
                            for ci, (pp, c0, w_) in enumerate(pch):
                                dp = stats.tile([128, 1], FP, tag=f"dnm{ci}",
                                                name=f"dnm{ci}")
                                nc.scalar.activation(
                                    probs[:, c0:c0 + w_], pp[:, :w_], AF.Exp,
                                    bias=nalib_t[:, head, ib:ib + 1],
                                    scale=1.0, accum_out=dp)
                                dparts.append(dp)
                            if len(dparts) == 2:
                                nc.vector.tensor_add(dparts[0], dparts[0],
                                                     dparts[1])
                            invd = stats.tile([128, 1], FP, tag="invd")
                            nc.vector.reciprocal(invd, dparts[0])
                            dn = workC.tile([128, 128], BF, tag="dn", bufs=5)
                            nc.vector.tensor_scalar_mul(dn, ident, invd)
                            probs_g.append(probs)
                            dn_g.append(dn)

                        pctx = psx.tile([128, 512], FP, tag="acc")
                        njc = 4 * g + 4
                        for jc in range(njc):
                            ib0 = max(jc, 4 * g)
                            nblk = 4 * g + 4 - ib0
                            pw = psw.tile([128, 512], FP, tag="wt")
                            for k, ib in enumerate(range(ib0, 4 * g + 4)):
                                nc.tensor.matmul(
                                    pw[:, k * 128:(k + 1) * 128],
                                    probs_g[ib - 4 * g][:, jc * 128:(jc + 1) * 128],
                                    dn_g[ib - 4 * g],
                                    start=True, stop=True)
                            wts = workC.tile([128, 512], BF, tag="wts", bufs=3)
                            nc.vector.tensor_copy(wts[:, :nblk * 128],
                                                  pw[:, :nblk * 128])
                            off = (ib0 - 4 * g) * 128
                            nc.tensor.matmul(
                                pctx[:, off:off + nblk * 128],
                                v_t[:, jc, head * HD:(head + 1) * HD],
                                wts[:, :nblk * 128],
                                start=(jc == 0), stop=(jc == njc - 1),
                                skip_group_check=True)
                        nc.vector.tensor_copy(
                            ctxT[:, head, g * 512:(g + 1) * 512], pctx)

                    def dense_rs1(g):
                        """dense partial for rows [g*512,(g+1)*512) + RS1.

                        A final K=64 one-hot selector matmul adds res1 for
                        exactly the rows this core owns, so rs1_out is the
                        FINISHED attention row and the AllGather can chain
                        directly off the ReduceScatter."""
                        dsb = workC.tile([128, 4, H], BF, tag="densebf")
                        for sb4 in range(4):
                            blk = 4 * g + sb4
                            for fc in range(4):
                                pd = psx.tile([128, 512], FP, tag="acc")
                                for h in range(NHC):
                                    nc.tensor.matmul(
                                        pd,
                                        ctxT[:, h, blk * 128:(blk + 1) * 128],
                                        wd_t[:, h, fc * 512:(fc + 1) * 512],
                                        start=(h == 0), stop=False,
                                        skip_group_check=True)
                                nc.tensor.matmul(
                                    pd, rsel_t[:, sb4, :],
                                    r1c_t[:, g, fc * 512:(fc + 1) * 512],
                                    start=False, stop=True,
                                    skip_group_check=True)
                                if fc % 2 == 0:
                                    nc.vector.tensor_copy(
                                        dsb[:, sb4, fc * 512:(fc + 1) * 512],
                                        pd)
                                else:
                                    nc.scalar.copy(
                                        dsb[:, sb4, fc * 512:(fc + 1) * 512],
                                        pd)
                        nc.sync.dma_start(
                            out=rs1_in[g][:, :].rearrange("(b p) h -> p b h",
                                                          p=128),
                            in_=dsb)
                        nc.gpsimd.collective_compute(
                            "ReduceScatter", ALU.add, replica_groups=rg,
                            ins=[rs1_in[g].opt()], outs=[rs1_out[g].opt()])

                    # stage_e is pure-gpsimd, so it can follow its dense
                    # immediately: AG_g fires at the earliest possible
                    # moment and no compute queue ever blocks on it.
                    for idx, g in enumerate(GORDER):
                        for head in range(NHC):
                            attention_head(g, head)
                        dense_rs1(g)
                        stage_e(g)

            # ======== Stage F: MLP + chunked RS2; Stage G epilogue ========
            with (
                tc.tile_pool(name="mlpw", bufs=1) as mlpw,
                tc.tile_pool(name="workF", bufs=2) as workF,
                tc.tile_pool(name="pf", bufs=2, space="PSUM") as pf_pool,
                tc.tile_pool(name="pt3", bufs=2, space="PSUM") as pt3_pool,
                tc.tile_pool(name="pm", bufs=2, space="PSUM") as pm_pool,
            ):
                # first chunk's gather BEFORE the weights: it unblocks the
                # transposes; wf1's first half follows (FC1 needs it next),
                # then wf2 (needed ~27us later).
                g0 = GORDER[0]
                ytmp0 = workF.tile([128, 4, H], BF, tag="ytmp", bufs=1,
                                   name="ytmp0")
                nc.sync.dma_start(
                    out=ytmp0,
                    in_=ag_out[g0][:, :, :].rearrange(
                        "c r h -> (c r) h").rearrange(
                        "(b p) h -> p b h", p=128))
                wf1_t = mlpw.tile([128, NHCH, F1C], BF, tag="wf1")
                nc.sync.dma_start(
                    out=wf1_t[:, :, :F1C // 2],
                    in_=wf1[:, :F1C // 2].rearrange("(c p) f -> p c f", p=128))
                nc.sync.dma_start(
                    out=wf1_t[:, :, F1C // 2:],
                    in_=wf1[:, F1C // 2:].rearrange("(c p) f -> p c f", p=128))
                wf2_t = mlpw.tile([128, NF1, H], BF, tag="wf2")
                nc.sync.dma_start(
                    out=wf2_t,
                    in_=wf2[:, :].rearrange("(c p) f -> p c f", p=128))

                def stage_g(g):
                    """rows of chunk g: out = rs2 + attn(+fc2b).

                    rs2 read waits on the RS2_g collective -> gpsimd queue."""
                    rsb2 = workF.tile([CROWS, H], BF, tag="rs2blk")
                    nc.gpsimd.dma_start(out=rsb2, in_=rs2_out[g])
                    ob = workF.tile([CROWS, H], FP, tag="outblk", bufs=1)
                    nc.gpsimd.tensor_add(ob, rsb2, attn_c[g])
                    nc.gpsimd.dma_start(out=out[g * CROWS:(g + 1) * CROWS, :],
                                        in_=ob)

                def prep(g, ytmp_pre=None):
                    """gather + replicated LN2 + transpose -> yTs for chunk g.

                    Emitted one chunk ahead (between FC1 and FC2 of the
                    previous chunk) so the DVE/ACT LN work hides under the
                    previous chunk's matmuls."""
                    if ytmp_pre is not None:
                        ytmp = ytmp_pre
                    else:
                        ytmp = workF.tile([128, 4, H], BF, tag="ytmp", bufs=1)
                        nc.gpsimd.dma_start(
                            out=ytmp,
                            in_=ag_out[g][:, :, :].rearrange(
                                "c r h -> (c r) h").rearrange(
                                "(b p) h -> p b h", p=128))
                    for b4 in range(4):
                        rstd, nmurs = ln_rowstats(ytmp[:, b4, :])
                        nc.scalar.activation(ytmp[:, b4, :], ytmp[:, b4, :],
                                             AF.Identity, bias=nmurs,
                                             scale=rstd)
                    yTs = workF.tile([128, NHCH, 512], BF, tag="yTs", bufs=2)
                    for hc in range(NHCH):
                        pt3 = pt3_pool.tile([128, 512], BF, tag="mmT3")
                        for b4 in range(4):
                            nc.tensor.transpose(
                                pt3[:, b4 * 128:(b4 + 1) * 128],
                                ytmp[:, b4, hc * 128:(hc + 1) * 128],
                                ident)
                        if hc % 2 == 0:
                            nc.vector.tensor_copy(yTs[:, hc, :], pt3)
                        else:
                            nc.scalar.copy(yTs[:, hc, :], pt3)
                    return yTs

                yTs_cur = prep(GORDER[0], ytmp_pre=ytmp0)
                for idx, g in enumerate(GORDER):
                    # fetch this core's finished attn rows (residual-2) and
                    # fold fc2b in (overlapped, off the tail)
                    nc.gpsimd.dma_start(out=attn_c[g], in_=rs1_out[g])
                    nc.gpsimd.tensor_add(attn_c[g], attn_c[g],
                                         fc2b_t[0:CROWS, :])
                    # FC1: hdnT[f1, seq-chunk] = gelu(Wf1 @ yT + b)
                    hdnT = workF.tile([128, NF1, 512], BF, tag="hdnT")
                    for f1c in range(NF1):
                        pf = pf_pool.tile([128, 512], FP, tag="mmf1")
                        for hc in range(NHCH):
                            nc.tensor.matmul(
                                pf, wf1_t[:, hc, f1c * 128:(f1c + 1) * 128],
                                yTs_cur[:, hc, :],
                                start=(hc == 0), stop=(hc == NHCH - 1))
                        nc.scalar.activation(
                            hdnT[:, f1c, :], pf,
                            AF.Identity if SIM_MODE else AF.Gelu_apprx_tanh,
                            bias=bf1_t[:, f1c:f1c + 1], scale=1.0)
                    # FC2 partial + RS2_g
                    for sb4 in range(4):
                        msb = workF.tile([128, H], BF, tag="mlpbf", bufs=2)
                        for half in range(2):
                            pm = pm_pool.tile([128, 1024], FP, tag="mmf2")
                            for f1c in range(NF1):
                                st = hdnT[:, f1c, sb4 * 128:(sb4 + 1) * 128]
                                for fc in range(2):
                                    f0 = half * 1024 + fc * 512
                                    nc.tensor.matmul(
                                        pm[:, fc * 512:(fc + 1) * 512],
                                        st, wf2_t[:, f1c, f0:f0 + 512],
                                        start=(f1c == 0),
                                        stop=(f1c == NF1 - 1),
                                        skip_group_check=True)
                            if half == 0:
                                nc.vector.tensor_copy(
                                    msb[:, half * 1024:(half + 1) * 1024], pm)
                            else:
                                nc.scalar.copy(
                                    msb[:, half * 1024:(half + 1) * 1024], pm)
                        nc.sync.dma_start(
                            out=rs2_in[g][sb4 * 128:(sb4 + 1) * 128, :],
                            in_=msb)
                    nc.gpsimd.collective_compute(
                        "ReduceScatter", ALU.add, replica_groups=rg,
                        ins=[rs2_in[g].opt()], outs=[rs2_out[g].opt()])
                    if idx >= 1:
                        stage_g(GORDER[idx - 1])
                    # next chunk's gather/LN2/transpose: AFTER this chunk's
                    # matmuls in the PE stream, so a late AllGather can only
                    # stall work that genuinely needs it.
                    if idx + 1 < len(GORDER):
                        yTs_cur = prep(GORDER[idx + 1])
                stage_g(GORDER[-1])
    nc.compile()
    return nc


def _host_prep(inputs):
    """Slice/fold weights per core. Returns list of per-core input maps."""
    bf16 = ml_dtypes.bfloat16
    hs = np.asarray(inputs["hidden_states"], np.float32).reshape(S, H)
    g1 = np.asarray(inputs["ln1_g"], np.float32)
    b1 = np.asarray(inputs["ln1_b"], np.float32)
    qkv_w = np.asarray(inputs["qkv_w"], np.float32)
    qkv_b = np.asarray(inputs["qkv_b"], np.float32)
    dense_w = np.asarray(inputs["dense_w"], np.float32)
    dense_b = np.asarray(inputs["dense_b"], np.float32)
    g2 = np.asarray(inputs["ln2_g"], np.float32)
    b2 = np.asarray(inputs["ln2_b"], np.float32)
    fc1_w = np.asarray(inputs["fc1_w"], np.float32)
    fc1_b = np.asarray(inputs["fc1_b"], np.float32)
    fc2_w = np.asarray(inputs["fc2_w"], np.float32)
    fc2_b = np.asarray(inputs["fc2_b"], np.float32)
    alibi = np.asarray(inputs["alibi"], np.float32).reshape(NH, S)

    inv = 1.0 / np.sqrt(np.float32(HD))
    hs_bf = hs.astype(bf16)

    # v-bias contribution to the dense output (softmax rows sum to 1):
    # ctx = ctx_nobv + bv  =>  dense += bv_cat @ dense_w.T  (fold into res1)
    bv_cat = np.zeros(H, np.float32)
    for h in range(NH):
        vr = qkv_w[h * 3 * HD + 2 * HD:h * 3 * HD + 3 * HD, :]
        bv_cat[h * HD:(h + 1) * HD] = (
            qkv_b[h * 3 * HD + 2 * HD:h * 3 * HD + 3 * HD] + vr @ b1)
    dense_b_eff = dense_b + bv_cat @ dense_w.T

    in_maps = []
    for c in range(NCORE):
        heads = [NHC * c + i for i in range(NHC)]
        wqk_cols, bqk_rows, wv_cols = [], [], []
        for h in heads:
            qr = qkv_w[h * 3 * HD:h * 3 * HD + HD, :]
            kr = qkv_w[h * 3 * HD + HD:h * 3 * HD + 2 * HD, :]
            vr = qkv_w[h * 3 * HD + 2 * HD:h * 3 * HD + 3 * HD, :]
            qb = qkv_b[h * 3 * HD:h * 3 * HD + HD] + qr @ b1
            kb = qkv_b[h * 3 * HD + HD:h * 3 * HD + 2 * HD] + kr @ b1
            wqk_cols.append((qr * g1[None, :]).T * inv)
            wqk_cols.append((kr * g1[None, :]).T)
            bqk_rows.append(qb * inv)
            bqk_rows.append(kb)
            wv_cols.append((vr * g1[None, :]).T)
        rows_c = np.concatenate(
            [np.arange(g * GROWS + c * CROWS, g * GROWS + (c + 1) * CROWS)
             for g in range(G)])
        # one-hot selector: within a 512-row chunk, this core owns rows
        # [c*64, (c+1)*64) -> block b4=c//2, partitions (c%2)*64 + k
        rsel_np = np.zeros((CROWS, 4, 128), np.float32)
        for k in range(CROWS):
            rsel_np[k, c // 2, (c % 2) * CROWS + k] = 1.0
        rsel_c = np.ascontiguousarray(rsel_np.reshape(CROWS, 512)).astype(bf16)
        alibi_c = alibi[heads[0]:heads[-1] + 1, :]          # [NHC, S]
        # coarse part: multiples of 8 (exact in bf16 up to 2048);
        # fine part in [-4, 4] (bf16 rounding ~2^-8 relative).
        acoarse = (8.0 * np.round(alibi_c / 8.0)).astype(bf16)
        afine = (alibi_c - acoarse.astype(np.float32)).astype(bf16)
        # the exp bias is the exact fp32 negation of the on-device sum
        nalib_c = -(acoarse.astype(np.float32) + afine.astype(np.float32))
        in_maps.append({
            "hid": hs_bf,
            "wqk": np.ascontiguousarray(
                np.concatenate(wqk_cols, axis=1)).astype(bf16),
            "bqk": np.ascontiguousarray(np.stack(bqk_rows, axis=0)),
            "wv": np.ascontiguousarray(
                np.concatenate(wv_cols, axis=1)).astype(bf16),
            "alibsp": np.ascontiguousarray(
                np.stack([acoarse.reshape(-1), afine.reshape(-1)], axis=0)),
            "nalib": np.ascontiguousarray(nalib_c.reshape(NHC, NSB, 128)),
            "wd": np.ascontiguousarray(
                dense_w[:, heads[0] * HD:(heads[-1] + 1) * HD].T).astype(bf16),
            "res1": np.ascontiguousarray(
                hs[rows_c, :] + dense_b_eff[None, :]).astype(bf16),
            "rsel": rsel_c,
            "wf1": np.ascontiguousarray(
                (fc1_w[c * F1C:(c + 1) * F1C, :] * g2[None, :]).T).astype(bf16),
            "bf1": np.ascontiguousarray(
                (fc1_b[c * F1C:(c + 1) * F1C]
                 + fc1_w[c * F1C:(c + 1) * F1C, :] @ b2
                 ).reshape(NF1, 128)),
            "wf2": np.ascontiguousarray(
                fc2_w[:, c * F1C:(c + 1) * F1C].T).astype(bf16),
            "fc2b": fc2_b,
        })
    return in_maps


def _assemble(results) -> np.ndarray:
    """Reassemble the strided row ownership into the full [B, S, H]."""
    full = np.empty((S, H), np.float32)
    for c in range(NCORE):
        shard = np.asarray(results[c]["out"], np.float32)  # [SSH, H]
        for g in range(G):
            full[g * GROWS + c * CROWS:g * GROWS + (c + 1) * CROWS, :] = \
                shard[g * CROWS:(g + 1) * CROWS, :]
    return full.reshape(B, S, H)


_CACHED_NC = None


def kernel(**inputs) -> np.ndarray:
    global _CACHED_NC
    in_maps = _host_prep(inputs)
    if _CACHED_NC is None:
        _CACHED_NC = build_program()
    res = run_bass_kernel_spmd(_CACHED_NC, in_maps, list(range(NCORE)))
    return _assemble(res.results)


# revision 40
# speedup vs baseline: 1.0247x; 1.0051x over previous
"""BloomBlock (B=1, S=2048, H=2048, NH=16) on 8 Trainium2 NeuronCores.

Megatron tensor-parallel: each core owns 2 attention heads and 1024 rows of
the 8192-wide MLP. LN1 (+transpose) is replicated; attention/MLP partial
sums are reduce-scattered in bf16; LN2 runs on the local sequence slices;
normalized activations are all-gathered for the MLP.

v2 (throughput rework):
  * All three collectives are CHUNKED over 4 sequence groups of 512 rows
    and issued as soon as their producer chunk finishes, so they overlap
    attention / MLP compute instead of idling every engine. Row ownership
    becomes strided: core c owns global rows {g*512 + c*64 + r}; the host
    reassembles.
  * Alibi and the causal mask are accumulated INTO the score PSUM by the
    tensor engine (a K=2 rank-2 matmul with ones against a bf16
    coarse+fine alibi split, and a transposed-causal-mask matmul), so no
    vector-engine pass over the [128, jw] scores is needed; exp reads
    PSUM directly.
  * Softmax drops the row-max pass: exp uses bias = -alibi[row] (alibi is
    monotonically increasing along the row, scores are O(5), so the
    exponent is bounded above by ~5 and the diagonal term keeps the
    denominator away from 0). The bias is the exact fp32 negation of the
    coarse+fine sum, so the diagonal exponent is exactly 0.
  * LN stats via bn_stats/bn_aggr (one DVE pass instead of reduce_sum +
    Square-accumulate).
  * DMAs consolidated (one descriptor per weight matrix, 2-D/3-D access
    patterns) - the baseline issued 690 DMAs at ~0.7us Sync issue each.
  * PSUM->SBUF copies widened and balanced across Vector/Scalar engines.
  * hidden_states streamed in bf16 (the fp32 residual arrives via res1);
    the v bias is folded into res1 on the host (softmax rows sum to 1).
"""
import sys

for _p in ("/opt/trn_rl_repo",):
    if _p not in sys.path:
        sys.path.insert(0, _p)

import numpy as np
import ml_dtypes

import concourse.bass as bass
from concourse import bacc
import concourse.mybir as mybir
import concourse.tile as tile
from concourse.bass_utils import run_bass_kernel_spmd
from concourse.masks import make_identity, make_lower_triangular

AF = mybir.ActivationFunctionType
ALU = mybir.AluOpType
AX = mybir.AxisListType

B, S, H, NH, HD = 1, 2048, 2048, 16, 128
NCORE = 8
NHC = NH // NCORE          # heads per core = 2
SSH = S // NCORE           # rows owned per core = 256
F1 = 4 * H                 # 8192
F1C = F1 // NCORE          # 1024
NSB = S // 128             # 16 s-blocks
NHCH = H // 128            # 16 h-chunks
NF1 = F1C // 128           # 8
EPS = 1e-5
MASK_NEG = -1e30
G = 4                      # collective chunks (512 rows each)
GROWS = S // G             # 512
CROWS = GROWS // NCORE     # 64 rows per core per chunk

FP = mybir.dt.float32
BF = mybir.dt.bfloat16

SIM_MODE = False           # CoreSim lacks Gelu; use Identity there
PANEL = 512                # stage-A transpose/QKV panel width
NPANEL = S // PANEL        # 4
GORDER = [0, 1, 2, 3]      # ascending: RS1_0/AG_0 fire early, F starts hole-free


def build_program():
    nc = bacc.Bacc("TRN2", target_bir_lowering=False, debug=False,
                   enable_asserts=False, num_devices=NCORE)

    # ---------------- I/O ----------------
    hid = nc.declare_dram_parameter("hid", [S, H], BF, isOutput=False)
    wqk = nc.declare_dram_parameter("wqk", [H, 4 * HD], BF, isOutput=False)
    bqk = nc.declare_dram_parameter("bqk", [4, HD], FP, isOutput=False)
    wv = nc.declare_dram_parameter("wv", [H, NHC * HD], BF, isOutput=False)
    alibsp = nc.declare_dram_parameter("alibsp", [2, NHC * S], BF,
                                       isOutput=False)
    nalib = nc.declare_dram_parameter("nalib", [NHC, NSB, 128], FP,
                                      isOutput=False)
    wd = nc.declare_dram_parameter("wd", [NHC * HD, H], BF, isOutput=False)
    res1 = nc.declare_dram_parameter("res1", [SSH, H], BF, isOutput=False)
    rsel = nc.declare_dram_parameter("rsel", [CROWS, 4 * 128], BF,
                                     isOutput=False)
    wf1 = nc.declare_dram_parameter("wf1", [H, F1C], BF, isOutput=False)
    bf1 = nc.declare_dram_parameter("bf1", [NF1, 128], FP, isOutput=False)
    wf2 = nc.declare_dram_parameter("wf2", [F1C, H], BF, isOutput=False)
    fc2b = nc.declare_dram_parameter("fc2b", [H], FP, isOutput=False)
    out = nc.declare_dram_parameter("out", [SSH, H], FP, isOutput=True)

    rg = [list(range(NCORE))]

    with tile.TileContext(nc) as tc:
        with (
            tc.tile_pool(name="dram", bufs=1, space="DRAM") as dram,
            tc.tile_pool(name="consts", bufs=1) as consts,
            tc.tile_pool(name="stats", bufs=4) as stats,
            tc.tile_pool(name="postp", bufs=1) as postp,
        ):
            # ------- collective bounce buffers (per chunk) -------
            rs1_in = [dram.tile([GROWS, H], BF, tag=f"rs1i{g}", name=f"rs1i{g}")
                      for g in range(G)]
            rs1_out = [dram.tile([CROWS, H], BF, tag=f"rs1o{g}",
                                 name=f"rs1o{g}") for g in range(G)]
            ag_out = [dram.tile([NCORE, CROWS, H], BF, tag=f"ago{g}",
                                name=f"ago{g}", addr_space="Shared")
                      for g in range(G)]
            rs2_in = [dram.tile([GROWS, H], BF, tag=f"rs2i{g}", name=f"rs2i{g}")
                      for g in range(G)]
            rs2_out = [dram.tile([CROWS, H], BF, tag=f"rs2o{g}",
                                 name=f"rs2o{g}") for g in range(G)]

            # ------------ constants (non-DMA first; DMAs after hid) ------
            ident = consts.tile([128, 128], BF, tag="ident")
            make_identity(nc, ident)
            # transposed causal mask: cmT[a,b] = MASK_NEG iff a > b, so
            # (cmT.T @ I)[i,j] = MASK_NEG iff j > i  (strictly-future).
            cmT = consts.tile([128, 128], BF, tag="cmT")
            make_lower_triangular(nc, cmT, val=MASK_NEG, diag=False)
            ones2 = consts.tile([2, 128], BF, tag="ones2")
            nc.vector.memset(ones2, 1.0)
            eps_t = consts.tile([128, 1], FP, tag="eps")
            nc.vector.memset(eps_t, EPS)
            bqk_t = consts.tile([128, 4], FP, tag="bqk")
            nalib_t = consts.tile([128, NHC, NSB], FP, tag="nalib")
            alibsp_t = consts.tile([2, NHC * S], BF, tag="alibsp")
            bf1_t = consts.tile([128, NF1], FP, tag="bf1")
            fc2b_t = consts.tile([128, H], FP, tag="fc2b")

            def load_consts(fstage):
                nc.sync.dma_start(out=bqk_t,
                                  in_=bqk[:, :].rearrange("b p -> p b"))
                nc.sync.dma_start(
                    out=nalib_t,
                    in_=nalib[:, :, :].rearrange("h b p -> p h b"))
                nc.sync.dma_start(out=alibsp_t, in_=alibsp[:, :])
                nc.sync.dma_start(out=bf1_t,
                                  in_=bf1[:, :].rearrange("b p -> p b"))
                nc.sync.dma_start(out=fstage[0:1, :], in_=fc2b[None, :])
                nc.gpsimd.partition_broadcast(fc2b_t, fstage[0:1, :])

            # fp32 attn rows (LN2 input + residual-2), per chunk; spans C..G
            attn_c = [postp.tile([CROWS, H], BF, tag=f"attn{g}",
                                 name=f"attn{g}") for g in range(G)]
            # stage-E scratch; spans C..F (E of the last groups is emitted
            # inside the stage-F stream)
            epool = postp

            def ln_rowstats(src):
                """bn_stats mean/var over the free axis of src [P, H].

                Returns (rstd, -mu*rstd) as [P, 1] fp32."""
                p = src.shape[0]
                bs = stats.tile([128, H // 512, 6], FP, tag="bnst")
                for c in range(H // 512):
                    nc.vector.bn_stats(bs[:p, c, :],
                                       src[:, c * 512:(c + 1) * 512])
                mv = stats.tile([128, 2], FP, tag="bnmv")
                nc.vector.bn_aggr(
                    mv[:p], bs[:p].rearrange("p c s -> p (c s)"))
                std = stats.tile([128, 1], FP, tag="std")
                nc.scalar.activation(std[:p], mv[:p, 1:2], AF.Sqrt,
                                     bias=eps_t[:p])
                rstd = stats.tile([128, 1], FP, tag="rstd")
                nc.vector.reciprocal(rstd[:p], std[:p])
                nmurs = stats.tile([128, 1], FP, tag="nmurs")
                nc.vector.tensor_mul(nmurs[:p], mv[:p, 0:1], rstd[:p])
                nc.vector.tensor_scalar_mul(nmurs[:p], nmurs[:p], -1.0)
                return rstd[:p], nmurs[:p]

            def stage_e(g):
                """AG of the finished (residual-included) attn rows.

                res1 was folded into the dense partial by the selector
                matmul, so rs1_out IS the attention output: the AllGather
                chains directly off the ReduceScatter on the CC stream
                with no intermediate compute."""
                nc.gpsimd.collective_compute(
                    "AllGather", ALU.bypass, replica_groups=rg,
                    ins=[rs1_out[g].opt()], outs=[ag_out[g].opt()])

            # ==== persistent attention activations (stages A..C) ====
            with tc.tile_pool(name="attnp", bufs=1) as attnp:
                qkT = attnp.tile([128, NHC, 2, S], BF, tag="qkT")
                v_t = attnp.tile([128, NSB, NHC * HD], BF, tag="v")
                ctxT = attnp.tile([128, NHC, S], BF, tag="ctxT")
                wd_t = attnp.tile([128, NHC, H], BF, tag="wd")
                r1c_t = attnp.tile([CROWS, G, H], BF, tag="r1c")
                rsel_t = attnp.tile([CROWS, 4, 128], BF, tag="rsel")

                # ---- Stage A: hid DMA, LN1, transpose, QKV per panel ----
                with (
                    tc.tile_pool(name="wpoolA", bufs=1) as wpoolA,
                    tc.tile_pool(name="workA", bufs=2) as workA,
                    tc.tile_pool(name="pA_t", bufs=2, space="PSUM") as pA_t,
                    tc.tile_pool(name="pA_qk", bufs=2, space="PSUM") as pA_qk,
                    tc.tile_pool(name="pA_v", bufs=2, space="PSUM") as pA_v,
                ):
                    hbs = {}
                    for sb in range(4):  # panel-0 rows first
                        hb = workA.tile([128, H], BF, tag="hidblk", bufs=4,
                                        name=f"hb{sb}")
                        nc.sync.dma_start(
                            out=hb, in_=hid[sb * 128:(sb + 1) * 128, :])
                        hbs[sb] = hb
                    fstage = wpoolA.tile([128, H], FP, tag="fc2bstage")
                    load_consts(fstage)
                    wqk_t = wpoolA.tile([128, NHCH, 4 * HD], BF, tag="wqk")
                    nc.sync.dma_start(
                        out=wqk_t,
                        in_=wqk[:, :].rearrange("(c p) f -> p c f", p=128))
                    wv_t = wpoolA.tile([128, NHCH, NHC * HD], BF, tag="wv")
                    nc.sync.dma_start(
                        out=wv_t,
                        in_=wv[:, :].rearrange("(c p) f -> p c f", p=128))
                    for sb in range(4, NSB):
                        hb = workA.tile([128, H], BF, tag="hidblk", bufs=4,
                                        name=f"hb{sb}")
                        nc.sync.dma_start(
                            out=hb, in_=hid[sb * 128:(sb + 1) * 128, :])
                        hbs[sb] = hb
                    nc.sync.dma_start(
                        out=wd_t,
                        in_=wd[:, :].rearrange("(c p) f -> p c f", p=128))
                    nc.sync.dma_start(
                        out=r1c_t,
                        in_=res1[:, :].rearrange("(g r) h -> r g h", r=CROWS))
                    nc.sync.dma_start(
                        out=rsel_t,
                        in_=rsel[:, :].rearrange("k (b p) -> k b p", p=128))

                    for p in range(NPANEL):
                        xhat_blocks = []
                        for sb4 in range(PANEL // 128):
                            sb = p * (PANEL // 128) + sb4
                            rstd, nmurs = ln_rowstats(hbs[sb])
                            xh = workA.tile([128, H], BF, tag="xhat", bufs=4)
                            nc.scalar.activation(xh, hbs[sb], AF.Identity,
                                                 bias=nmurs, scale=rstd)
                            xhat_blocks.append(xh)

                        xT = workA.tile([128, NHCH, PANEL], BF, tag="xT")
                        for hc in range(NHCH):
                            pt = pA_t.tile([128, PANEL], BF, tag="mmT")
                            for sb4 in range(PANEL // 128):
                                nc.tensor.transpose(
                                    pt[:, sb4 * 128:(sb4 + 1) * 128],
                                    xhat_blocks[sb4][:, hc * 128:(hc + 1) * 128],
                                    ident)
                            if hc % 2 == 0:
                                nc.vector.tensor_copy(xT[:, hc, :], pt)
                            else:
                                nc.scalar.copy(xT[:, hc, :], pt)

                        for fb in range(4):  # q_h0, k_h0, q_h1, k_h1
                            pq = pA_qk.tile([128, PANEL], FP, tag="mmqk")
                            for hc in range(NHCH):
                                nc.tensor.matmul(
                                    pq, wqk_t[:, hc, fb * 128:(fb + 1) * 128],
                                    xT[:, hc, :],
                                    start=(hc == 0), stop=(hc == NHCH - 1))
                            head, isk = fb // 2, fb % 2
                            nc.scalar.activation(
                                qkT[:, head, isk, p * PANEL:(p + 1) * PANEL],
                                pq, AF.Identity, bias=bqk_t[:, fb:fb + 1],
                                scale=1.0)

                        for sb4 in range(PANEL // 128):
                            blk = p * (PANEL // 128) + sb4
                            pv = pA_v.tile([128, NHC * HD], FP, tag="mmv")
                            for hc in range(NHCH):
                                nc.tensor.matmul(
                                    pv, xT[:, hc, sb4 * 128:(sb4 + 1) * 128],
                                    wv_t[:, hc, :],
                                    start=(hc == 0), stop=(hc == NHCH - 1))
                            nc.vector.tensor_copy(v_t[:, blk, :], pv)

                # ==== Stage C/D/E: attention + dense + chunked RS1/AG ====
                with (
                    tc.tile_pool(name="workC", bufs=2) as workC,
                    tc.tile_pool(name="psc", bufs=2, space="PSUM") as psc,
                    tc.tile_pool(name="psw", bufs=2, space="PSUM") as psw,
                    tc.tile_pool(name="psx", bufs=2, space="PSUM") as psx,
                ):
                    def attention_head(g, head):
                        """scores/softmax/ctx for q-rows [g*512,(g+1)*512)."""
                        probs_g, dn_g = [], []
                        for ib in range(4 * g, 4 * g + 4):
                            jw = (ib + 1) * 128
                            qblk = qkT[:, head, 0, ib * 128:(ib + 1) * 128]
                            nch = (jw + 1023) // 1024
                            pch = []
                            for cc in range(nch):
                                c0 = cc * 1024
                                w_ = min(1024, jw - c0)
                                pp = psc.tile([128, 1024], FP, tag="sc")
                                pch.append((pp, c0, w_))
                            # pass 1: scores (stationary q-block)
                            for pp, c0, w_ in pch:
                                for s0 in range(0, w_, 512):
                                    sw = min(512, w_ - s0)
                                    nc.tensor.matmul(
                                        pp[:, s0:s0 + sw], qblk,
                                        qkT[:, head, 1, c0 + s0:c0 + s0 + sw],
                                        start=True, stop=False,
                                        skip_group_check=True)
                            # pass 2: + alibi (rank-2: coarse+fine rows)
                            for pp, c0, w_ in pch:
                                for s0 in range(0, w_, 512):
                                    sw = min(512, w_ - s0)
                                    j0 = head * S + c0 + s0
                                    isdiag = (c0 + s0 + sw == jw)
                                    nc.tensor.matmul(
                                        pp[:, s0:s0 + sw], ones2,
                                        alibsp_t[:, j0:j0 + sw],
                                        start=False, stop=not isdiag,
                                        skip_group_check=True)
                            # pass 3: causal mask on the diagonal block
                            pp, c0, w_ = pch[-1]
                            d0 = (jw - 128) - c0
                            nc.tensor.matmul(
                                pp[:, d0:d0 + 128], cmT, ident,
                                start=False, stop=True,
                                skip_group_check=True)
                            # exp (PSUM-read) + denominators
                            probs = workC.tile([128, S], BF, tag="probs",
                                               bufs=4)
                            dparts = []
                            for ci, (pp, c0, w_) in enumerate(pch):
                                dp = stats.tile([128, 1], FP, tag=f"dnm{ci}",
                                                name=f"dnm{ci}")
                                nc.scalar.activation(
                                    probs[:, c0:c0 + w_], pp[:, :w_], AF.Exp,
                                    bias=nalib_t[:, head, ib:ib + 1],
                                    scale=1.0, accum_out=dp)
                                dparts.append(dp)
                            if len(dparts) == 2:
                                nc.vector.tensor_add(dparts[0], dparts[0],
                                                     dparts[1])
                            invd = stats.tile([128, 1], FP, tag="invd")
                            nc.vector.reciprocal(invd, dparts[0])
                            dn = workC.tile([128, 128], BF, tag="dn", bufs=5)
                            nc.vector.tensor_scalar_mul(dn, ident, invd)
                            probs_g.append(probs)
                            dn_g.append(dn)

                        pctx = psx.tile([128, 512], FP, tag="acc")
                        njc = 4 * g + 4

                        def wt_fill(jc):
                            """probs.T x diag(1/denom) for key-block jc."""
                            ib0 = max(jc, 4 * g)
                            nblk = 4 * g + 4 - ib0
                            pw = psw.tile([128, 512], FP, tag="wt")
                            for k, ib in enumerate(range(ib0, 4 * g + 4)):
                                nc.tensor.matmul(
                                    pw[:, k * 128:(k + 1) * 128],
                                    probs_g[ib - 4 * g][:, jc * 128:(jc + 1) * 128],
                                    dn_g[ib - 4 * g],
                                    start=True, stop=True)
                            wts = workC.tile([128, 512], BF, tag="wts",
                                             bufs=3)
                            nc.vector.tensor_copy(wts[:, :nblk * 128],
                                                  pw[:, :nblk * 128])
                            return wts, ib0, nblk

                        # software-pipelined: fill pw(jc+1) on the PE while
                        # ctx(jc) waits for its DVE wts copy, instead of a
                        # hard PE->DVE->PE round-trip every iteration.
                        cur = wt_fill(0)
                        for jc in range(njc):
                            nxt = wt_fill(jc + 1) if jc + 1 < njc else None
                            wts, ib0, nblk = cur
                            off = (ib0 - 4 * g) * 128
                            nc.tensor.matmul(
                                pctx[:, off:off + nblk * 128],
                                v_t[:, jc, head * HD:(head + 1) * HD],
                                wts[:, :nblk * 128],
                                start=(jc == 0), stop=(jc == njc - 1),
                                skip_group_check=True)
                            cur = nxt
                        nc.vector.tensor_copy(
                            ctxT[:, head, g * 512:(g + 1) * 512], pctx)

                    def dense_rs1(g):
                        """dense partial for rows [g*512,(g+1)*512) + RS1.

                        A final K=64 one-hot selector matmul adds res1 for
                        exactly the rows this core owns, so rs1_out is the
                        FINISHED attention row and the AllGather can chain
                        directly off the ReduceScatter."""
                        dsb = workC.tile([128, 4, H], BF, tag="densebf")
                        for sb4 in range(4):
                            blk = 4 * g + sb4
                            for fc in range(4):
                                pd = psx.tile([128, 512], FP, tag="acc")
                                for h in range(NHC):
                                    nc.tensor.matmul(
                                        pd,
                                        ctxT[:, h, blk * 128:(blk + 1) * 128],
                                        wd_t[:, h, fc * 512:(fc + 1) * 512],
                                        start=(h == 0), stop=False,
                                        skip_group_check=True)
                                nc.tensor.matmul(
                                    pd, rsel_t[:, sb4, :],
                                    r1c_t[:, g, fc * 512:(fc + 1) * 512],
                                    start=False, stop=True,
                                    skip_group_check=True)
                                if fc % 2 == 0:
                                    nc.vector.tensor_copy(
                                        dsb[:, sb4, fc * 512:(fc + 1) * 512],
                                        pd)
                                else:
                                    nc.scalar.copy(
                                        dsb[:, sb4, fc * 512:(fc + 1) * 512],
                                        pd)
                        nc.sync.dma_start(
                            out=rs1_in[g][:, :].rearrange("(b p) h -> p b h",
                                                          p=128),
                            in_=dsb)
                        nc.gpsimd.collective_compute(
                            "ReduceScatter", ALU.add, replica_groups=rg,
                            ins=[rs1_in[g].opt()], outs=[rs1_out[g].opt()])

                    # stage_e is pure-gpsimd, so it can follow its dense
                    # immediately: AG_g fires at the earliest possible
                    # moment and no compute queue ever blocks on it.
                    for idx, g in enumerate(GORDER):
                        for head in range(NHC):
                            attention_head(g, head)
                        dense_rs1(g)
                        stage_e(g)

            # ======== Stage F: MLP + chunked RS2; Stage G epilogue ========
            with (
                tc.tile_pool(name="mlpw", bufs=1) as mlpw,
                tc.tile_pool(name="workF", bufs=2) as workF,
                tc.tile_pool(name="pf", bufs=2, space="PSUM") as pf_pool,
                tc.tile_pool(name="pt3", bufs=2, space="PSUM") as pt3_pool,
                tc.tile_pool(name="pm", bufs=2, space="PSUM") as pm_pool,
            ):
                # first chunk's gather BEFORE the weights: it unblocks the
                # transposes; wf1's first half follows (FC1 needs it next),
                # then wf2 (needed ~27us later).
                g0 = GORDER[0]
                ytmp0 = workF.tile([128, 4, H], BF, tag="ytmp", bufs=1,
                                   name="ytmp0")
                nc.sync.dma_start(
                    out=ytmp0,
                    in_=ag_out[g0][:, :, :].rearrange(
                        "c r h -> (c r) h").rearrange(
                        "(b p) h -> p b h", p=128))
                wf1_t = mlpw.tile([128, NHCH, F1C], BF, tag="wf1")
                nc.sync.dma_start(
                    out=wf1_t[:, :, :F1C // 2],
                    in_=wf1[:, :F1C // 2].rearrange("(c p) f -> p c f", p=128))
                nc.sync.dma_start(
                    out=wf1_t[:, :, F1C // 2:],
                    in_=wf1[:, F1C // 2:].rearrange("(c p) f -> p c f", p=128))
                wf2_t = mlpw.tile([128, NF1, H], BF, tag="wf2")
                nc.sync.dma_start(
                    out=wf2_t,
                    in_=wf2[:, :].rearrange("(c p) f -> p c f", p=128))

                def stage_g(g):
                    """rows of chunk g: out = rs2 + attn(+fc2b).

                    rs2 read waits on the RS2_g collective -> gpsimd queue."""
                    rsb2 = workF.tile([CROWS, H], BF, tag="rs2blk")
                    nc.gpsimd.dma_start(out=rsb2, in_=rs2_out[g])
                    ob = workF.tile([CROWS, H], FP, tag="outblk", bufs=1)
                    nc.gpsimd.tensor_add(ob, rsb2, attn_c[g])
                    nc.gpsimd.dma_start(out=out[g * CROWS:(g + 1) * CROWS, :],
                                        in_=ob)

                def prep(g, ytmp_pre=None):
                    """gather + replicated LN2 + transpose -> yTs for chunk g.

                    Emitted one chunk ahead (between FC1 and FC2 of the
                    previous chunk) so the DVE/ACT LN work hides under the
                    previous chunk's matmuls."""
                    if ytmp_pre is not None:
                        ytmp = ytmp_pre
                    else:
                        ytmp = workF.tile([128, 4, H], BF, tag="ytmp", bufs=1)
                        nc.gpsimd.dma_start(
                            out=ytmp,
                            in_=ag_out[g][:, :, :].rearrange(
                                "c r h -> (c r) h").rearrange(
                                "(b p) h -> p b h", p=128))
                    for b4 in range(4):
                        rstd, nmurs = ln_rowstats(ytmp[:, b4, :])
                        nc.scalar.activation(ytmp[:, b4, :], ytmp[:, b4, :],
                                             AF.Identity, bias=nmurs,
                                             scale=rstd)
                    yTs = workF.tile([128, NHCH, 512], BF, tag="yTs", bufs=2)
                    for hc in range(NHCH):
                        pt3 = pt3_pool.tile([128, 512], BF, tag="mmT3")
                        for b4 in range(4):
                            nc.tensor.transpose(
                                pt3[:, b4 * 128:(b4 + 1) * 128],
                                ytmp[:, b4, hc * 128:(hc + 1) * 128],
                                ident)
                        if hc % 2 == 0:
                            nc.vector.tensor_copy(yTs[:, hc, :], pt3)
                        else:
                            nc.scalar.copy(yTs[:, hc, :], pt3)
                    return yTs

                yTs_cur = prep(GORDER[0], ytmp_pre=ytmp0)
                for idx, g in enumerate(GORDER):
                    # fetch this core's finished attn rows (residual-2) and
                    # fold fc2b in (overlapped, off the tail)
                    nc.gpsimd.dma_start(out=attn_c[g], in_=rs1_out[g])
                    nc.gpsimd.tensor_add(attn_c[g], attn_c[g],
                                         fc2b_t[0:CROWS, :])
                    # FC1: hdnT[f1, seq-chunk] = gelu(Wf1 @ yT + b)
                    hdnT = workF.tile([128, NF1, 512], BF, tag="hdnT")
                    for f1c in range(NF1):
                        pf = pf_pool.tile([128, 512], FP, tag="mmf1")
                        for hc in range(NHCH):
                            nc.tensor.matmul(
                                pf, wf1_t[:, hc, f1c * 128:(f1c + 1) * 128],
                                yTs_cur[:, hc, :],
                                start=(hc == 0), stop=(hc == NHCH - 1))
                        nc.scalar.activation(
                            hdnT[:, f1c, :], pf,
                            AF.Identity if SIM_MODE else AF.Gelu_apprx_tanh,
                            bias=bf1_t[:, f1c:f1c + 1], scale=1.0)
                    # FC2 partial + RS2_g
                    for sb4 in range(4):
                        msb = workF.tile([128, H], BF, tag="mlpbf", bufs=2)
                        for half in range(2):
                            pm = pm_pool.tile([128, 1024], FP, tag="mmf2")
                            for f1c in range(NF1):
                                st = hdnT[:, f1c, sb4 * 128:(sb4 + 1) * 128]
                                for fc in range(2):
                                    f0 = half * 1024 + fc * 512
                                    nc.tensor.matmul(
                                        pm[:, fc * 512:(fc + 1) * 512],
                                        st, wf2_t[:, f1c, f0:f0 + 512],
                                        start=(f1c == 0),
                                        stop=(f1c == NF1 - 1),
                                        skip_group_check=True)
                            if half == 0:
                                nc.vector.tensor_copy(
                                    msb[:, half * 1024:(half + 1) * 1024], pm)
                            else:
                                nc.scalar.copy(
                                    msb[:, half * 1024:(half + 1) * 1024], pm)
                        nc.sync.dma_start(
                            out=rs2_in[g][sb4 * 128:(sb4 + 1) * 128, :],
                            in_=msb)
                    nc.gpsimd.collective_compute(
                        "ReduceScatter", ALU.add, replica_groups=rg,
                        ins=[rs2_in[g].opt()], outs=[rs2_out[g].opt()])
                    if idx >= 1:
                        stage_g(GORDER[idx - 1])
                    # next chunk's gather/LN2/transpose: AFTER this chunk's
                    # matmuls in the PE stream, so a late AllGather can only
                    # stall work that genuinely needs it.
                    if idx + 1 < len(GORDER):
                        yTs_cur = prep(GORDER[idx + 1])
                stage_g(GORDER[-1])
    nc.compile()
    return nc


def _host_prep(inputs):
    """Slice/fold weights per core. Returns list of per-core input maps."""
    bf16 = ml_dtypes.bfloat16
    hs = np.asarray(inputs["hidden_states"], np.float32).reshape(S, H)
    g1 = np.asarray(inputs["ln1_g"], np.float32)
    b1 = np.asarray(inputs["ln1_b"], np.float32)
    qkv_w = np.asarray(inputs["qkv_w"], np.float32)
    qkv_b = np.asarray(inputs["qkv_b"], np.float32)
    dense_w = np.asarray(inputs["dense_w"], np.float32)
    dense_b = np.asarray(inputs["dense_b"], np.float32)
    g2 = np.asarray(inputs["ln2_g"], np.float32)
    b2 = np.asarray(inputs["ln2_b"], np.float32)
    fc1_w = np.asarray(inputs["fc1_w"], np.float32)
    fc1_b = np.asarray(inputs["fc1_b"], np.float32)
    fc2_w = np.asarray(inputs["fc2_w"], np.float32)
    fc2_b = np.asarray(inputs["fc2_b"], np.float32)
    alibi = np.asarray(inputs["alibi"], np.float32).reshape(NH, S)

    inv = 1.0 / np.sqrt(np.float32(HD))
    hs_bf = hs.astype(bf16)

    # v-bias contribution to the dense output (softmax rows sum to 1):
    # ctx = ctx_nobv + bv  =>  dense += bv_cat @ dense_w.T  (fold into res1)
    bv_cat = np.zeros(H, np.float32)
    for h in range(NH):
        vr = qkv_w[h * 3 * HD + 2 * HD:h * 3 * HD + 3 * HD, :]
        bv_cat[h * HD:(h + 1) * HD] = (
            qkv_b[h * 3 * HD + 2 * HD:h * 3 * HD + 3 * HD] + vr @ b1)
    dense_b_eff = dense_b + bv_cat @ dense_w.T

    in_maps = []
    for c in range(NCORE):
        heads = [NHC * c + i for i in range(NHC)]
        wqk_cols, bqk_rows, wv_cols = [], [], []
        for h in heads:
            qr = qkv_w[h * 3 * HD:h * 3 * HD + HD, :]
            kr = qkv_w[h * 3 * HD + HD:h * 3 * HD + 2 * HD, :]
            vr = qkv_w[h * 3 * HD + 2 * HD:h * 3 * HD + 3 * HD, :]
            qb = qkv_b[h * 3 * HD:h * 3 * HD + HD] + qr @ b1
            kb = qkv_b[h * 3 * HD + HD:h * 3 * HD + 2 * HD] + kr @ b1
            wqk_cols.append((qr * g1[None, :]).T * inv)
            wqk_cols.append((kr * g1[None, :]).T)
            bqk_rows.append(qb * inv)
            bqk_rows.append(kb)
            wv_cols.append((vr * g1[None, :]).T)
        rows_c = np.concatenate(
            [np.arange(g * GROWS + c * CROWS, g * GROWS + (c + 1) * CROWS)
             for g in range(G)])
        # one-hot selector: within a 512-row chunk, this core owns rows
        # [c*64, (c+1)*64) -> block b4=c//2, partitions (c%2)*64 + k
        rsel_np = np.zeros((CROWS, 4, 128), np.float32)
        for k in range(CROWS):
            rsel_np[k, c // 2, (c % 2) * CROWS + k] = 1.0
        rsel_c = np.ascontiguousarray(rsel_np.reshape(CROWS, 512)).astype(bf16)
        alibi_c = alibi[heads[0]:heads[-1] + 1, :]          # [NHC, S]
        # coarse part: multiples of 8 (exact in bf16 up to 2048);
        # fine part in [-4, 4] (bf16 rounding ~2^-8 relative).
        acoarse = (8.0 * np.round(alibi_c / 8.0)).astype(bf16)
        afine = (alibi_c - acoarse.astype(np.float32)).astype(bf16)
        # the exp bias is the exact fp32 negation of the on-device sum
        nalib_c = -(acoarse.astype(np.float32) + afine.astype(np.float32))
        in_maps.append({
            "hid": hs_bf,
            "wqk": np.ascontiguousarray(
                np.concatenate(wqk_cols, axis=1)).astype(bf16),
            "bqk": np.ascontiguousarray(np.stack(bqk_rows, axis=0)),
            "wv": np.ascontiguousarray(
                np.concatenate(wv_cols, axis=1)).astype(bf16),
            "alibsp": np.ascontiguousarray(
                np.stack([acoarse.reshape(-1), afine.reshape(-1)], axis=0)),
            "nalib": np.ascontiguousarray(nalib_c.reshape(NHC, NSB, 128)),
            "wd": np.ascontiguousarray(
                dense_w[:, heads[0] * HD:(heads[-1] + 1) * HD].T).astype(bf16),
            "res1": np.ascontiguousarray(
                hs[rows_c, :] + dense_b_eff[None, :]).astype(bf16),
            "rsel": rsel_c,
            "wf1": np.ascontiguousarray(
                (fc1_w[c * F1C:(c + 1) * F1C, :] * g2[None, :]).T).astype(bf16),
            "bf1": np.ascontiguousarray(
                (fc1_b[c * F1C:(c + 1) * F1C]
                 + fc1_w[c * F1C:(c + 1) * F1C, :] @ b2
                 ).reshape(NF1, 128)),
            "wf2": np.ascontiguousarray(
                fc2_w[:, c * F1C:(c + 1) * F1C].T).astype(bf16),
            "fc2b": fc2_b,
        })
    return in_maps


def _assemble(results) -> np.ndarray:
    """Reassemble the strided row ownership into the full [B, S, H]."""
    full = np.empty((S, H), np.float32)
    for c in range(NCORE):
        shard = np.asarray(results[c]["out"], np.float32)  # [SSH, H]
        for g in range(G):
            full[g * GROWS + c * CROWS:g * GROWS + (c + 1) * CROWS, :] = \
                shard[g * CROWS:(g + 1) * CROWS, :]
    return full.reshape(B, S, H)


_CACHED_NC = None


def kernel(**inputs) -> np.ndarray:
    global _CACHED_NC
    in_maps = _host_prep(inputs)
    if _CACHED_NC is None:
        _CACHED_NC = build_program()
    res = run_bass_kernel_spmd(_CACHED_NC, in_maps, list(range(NCORE)))
    return _assemble(res.results)


# revision 41
# speedup vs baseline: 1.0595x; 1.0340x over previous
"""BloomBlock (B=1, S=2048, H=2048, NH=16) on 8 Trainium2 NeuronCores.

Megatron tensor-parallel: each core owns 2 attention heads and 1024 rows of
the 8192-wide MLP. LN1 (+transpose) is replicated; attention/MLP partial
sums are reduce-scattered in bf16; LN2 runs on the local sequence slices;
normalized activations are all-gathered for the MLP.

v2 (throughput rework):
  * All three collectives are CHUNKED over 4 sequence groups of 512 rows
    and issued as soon as their producer chunk finishes, so they overlap
    attention / MLP compute instead of idling every engine. Row ownership
    becomes strided: core c owns global rows {g*512 + c*64 + r}; the host
    reassembles.
  * Alibi and the causal mask are accumulated INTO the score PSUM by the
    tensor engine (a K=2 rank-2 matmul with ones against a bf16
    coarse+fine alibi split, and a transposed-causal-mask matmul), so no
    vector-engine pass over the [128, jw] scores is needed; exp reads
    PSUM directly.
  * Softmax drops the row-max pass: exp uses bias = -alibi[row] (alibi is
    monotonically increasing along the row, scores are O(5), so the
    exponent is bounded above by ~5 and the diagonal term keeps the
    denominator away from 0). The bias is the exact fp32 negation of the
    coarse+fine sum, so the diagonal exponent is exactly 0.
  * LN stats via bn_stats/bn_aggr (one DVE pass instead of reduce_sum +
    Square-accumulate).
  * DMAs consolidated (one descriptor per weight matrix, 2-D/3-D access
    patterns) - the baseline issued 690 DMAs at ~0.7us Sync issue each.
  * PSUM->SBUF copies widened and balanced across Vector/Scalar engines.
  * hidden_states streamed in bf16 (the fp32 residual arrives via res1);
    the v bias is folded into res1 on the host (softmax rows sum to 1).
"""
import sys

for _p in ("/opt/trn_rl_repo",):
    if _p not in sys.path:
        sys.path.insert(0, _p)

import numpy as np
import ml_dtypes

import concourse.bass as bass
from concourse import bacc
import concourse.mybir as mybir
import concourse.tile as tile
from concourse.bass_utils import run_bass_kernel_spmd
from concourse.masks import make_identity, make_lower_triangular

AF = mybir.ActivationFunctionType
ALU = mybir.AluOpType
AX = mybir.AxisListType

B, S, H, NH, HD = 1, 2048, 2048, 16, 128
NCORE = 8
NHC = NH // NCORE          # heads per core = 2
SSH = S // NCORE           # rows owned per core = 256
F1 = 4 * H                 # 8192
F1C = F1 // NCORE          # 1024
NSB = S // 128             # 16 s-blocks
NHCH = H // 128            # 16 h-chunks
NF1 = F1C // 128           # 8
EPS = 1e-5
MASK_NEG = -1e30
G = 4                      # collective chunks (512 rows each)
GROWS = S // G             # 512
CROWS = GROWS // NCORE     # 64 rows per core per chunk

FP = mybir.dt.float32
BF = mybir.dt.bfloat16

SIM_MODE = False           # CoreSim lacks Gelu; use Identity there
PANEL = 512                # stage-A transpose/QKV panel width
NPANEL = S // PANEL        # 4
GORDER = [0, 1, 2, 3]      # ascending: RS1_0/AG_0 fire early, F starts hole-free


def build_program():
    nc = bacc.Bacc("TRN2", target_bir_lowering=False, debug=False,
                   enable_asserts=False, num_devices=NCORE)

    # ---------------- I/O ----------------
    hid = nc.declare_dram_parameter("hid", [S, H], BF, isOutput=False)
    wqk = nc.declare_dram_parameter("wqk", [H, 4 * HD], BF, isOutput=False)
    bqk = nc.declare_dram_parameter("bqk", [4, HD], FP, isOutput=False)
    wv = nc.declare_dram_parameter("wv", [H, NHC * HD], BF, isOutput=False)
    alibsp = nc.declare_dram_parameter("alibsp", [2, NHC * S], BF,
                                       isOutput=False)
    nalib = nc.declare_dram_parameter("nalib", [NHC, NSB, 128], FP,
                                      isOutput=False)
    wd = nc.declare_dram_parameter("wd", [NHC * HD, H], BF, isOutput=False)
    res1 = nc.declare_dram_parameter("res1", [SSH, H], BF, isOutput=False)
    rsel = nc.declare_dram_parameter("rsel", [CROWS, 4 * 128], BF,
                                     isOutput=False)
    wf1 = nc.declare_dram_parameter("wf1", [H, F1C], BF, isOutput=False)
    bf1 = nc.declare_dram_parameter("bf1", [NF1, 128], FP, isOutput=False)
    wf2 = nc.declare_dram_parameter("wf2", [F1C, H], BF, isOutput=False)
    fc2b = nc.declare_dram_parameter("fc2b", [H], FP, isOutput=False)
    out = nc.declare_dram_parameter("out", [SSH, H], FP, isOutput=True)

    rg = [list(range(NCORE))]

    with tile.TileContext(nc) as tc:
        with (
            tc.tile_pool(name="dram", bufs=1, space="DRAM") as dram,
            tc.tile_pool(name="consts", bufs=1) as consts,
            tc.tile_pool(name="stats", bufs=4) as stats,
            tc.tile_pool(name="postp", bufs=1) as postp,
        ):
            # ------- collective bounce buffers (per chunk) -------
            rs1_in = [dram.tile([GROWS, H], BF, tag=f"rs1i{g}", name=f"rs1i{g}")
                      for g in range(G)]
            rs1_out = [dram.tile([CROWS, H], BF, tag=f"rs1o{g}",
                                 name=f"rs1o{g}") for g in range(G)]
            ag_out = [dram.tile([NCORE, CROWS, H], BF, tag=f"ago{g}",
                                name=f"ago{g}", addr_space="Shared")
                      for g in range(G)]
            rs2_in = [dram.tile([GROWS, H], BF, tag=f"rs2i{g}", name=f"rs2i{g}")
                      for g in range(G)]
            rs2_out = [dram.tile([CROWS, H], BF, tag=f"rs2o{g}",
                                 name=f"rs2o{g}") for g in range(G)]

            # ------------ constants (non-DMA first; DMAs after hid) ------
            ident = consts.tile([128, 128], BF, tag="ident")
            make_identity(nc, ident)
            # transposed causal mask: cmT[a,b] = MASK_NEG iff a > b, so
            # (cmT.T @ I)[i,j] = MASK_NEG iff j > i  (strictly-future).
            cmT = consts.tile([128, 128], BF, tag="cmT")
            make_lower_triangular(nc, cmT, val=MASK_NEG, diag=False)
            ones2 = consts.tile([2, 128], BF, tag="ones2")
            nc.vector.memset(ones2, 1.0)
            eps_t = consts.tile([128, 1], FP, tag="eps")
            nc.vector.memset(eps_t, EPS)
            bqk_t = consts.tile([128, 4], FP, tag="bqk")
            nalib_t = consts.tile([128, NHC, NSB], FP, tag="nalib")
            alibsp_t = consts.tile([2, NHC * S], BF, tag="alibsp")
            bf1_t = consts.tile([128, NF1], FP, tag="bf1")
            fc2b_t = consts.tile([128, H], FP, tag="fc2b")

            def load_consts(fstage):
                nc.sync.dma_start(out=bqk_t,
                                  in_=bqk[:, :].rearrange("b p -> p b"))
                nc.sync.dma_start(
                    out=nalib_t,
                    in_=nalib[:, :, :].rearrange("h b p -> p h b"))
                nc.sync.dma_start(out=alibsp_t, in_=alibsp[:, :])
                nc.sync.dma_start(out=bf1_t,
                                  in_=bf1[:, :].rearrange("b p -> p b"))
                nc.sync.dma_start(out=fstage[0:1, :], in_=fc2b[None, :])
                nc.gpsimd.partition_broadcast(fc2b_t, fstage[0:1, :])

            # fp32 attn rows (LN2 input + residual-2), per chunk; spans C..G
            attn_c = [postp.tile([CROWS, H], BF, tag=f"attn{g}",
                                 name=f"attn{g}") for g in range(G)]
            # stage-E scratch; spans C..F (E of the last groups is emitted
            # inside the stage-F stream)
            epool = postp

            def ln_rowstats(src):
                """bn_stats mean/var over the free axis of src [P, H].

                Returns (rstd, -mu*rstd) as [P, 1] fp32."""
                p = src.shape[0]
                bs = stats.tile([128, H // 512, 6], FP, tag="bnst")
                for c in range(H // 512):
                    nc.vector.bn_stats(bs[:p, c, :],
                                       src[:, c * 512:(c + 1) * 512])
                mv = stats.tile([128, 2], FP, tag="bnmv")
                nc.vector.bn_aggr(
                    mv[:p], bs[:p].rearrange("p c s -> p (c s)"))
                std = stats.tile([128, 1], FP, tag="std")
                nc.scalar.activation(std[:p], mv[:p, 1:2], AF.Sqrt,
                                     bias=eps_t[:p])
                rstd = stats.tile([128, 1], FP, tag="rstd")
                nc.vector.reciprocal(rstd[:p], std[:p])
                nmurs = stats.tile([128, 1], FP, tag="nmurs")
                nc.vector.tensor_mul(nmurs[:p], mv[:p, 0:1], rstd[:p])
                nc.vector.tensor_scalar_mul(nmurs[:p], nmurs[:p], -1.0)
                return rstd[:p], nmurs[:p]

            def stage_e(g):
                """AG of the finished (residual-included) attn rows.

                res1 was folded into the dense partial by the selector
                matmul, so rs1_out IS the attention output: the AllGather
                chains directly off the ReduceScatter on the CC stream
                with no intermediate compute."""
                nc.gpsimd.collective_compute(
                    "AllGather", ALU.bypass, replica_groups=rg,
                    ins=[rs1_out[g].opt()], outs=[ag_out[g].opt()])

            # ==== persistent attention activations (stages A..C) ====
            with tc.tile_pool(name="attnp", bufs=1) as attnp:
                qkT = attnp.tile([128, NHC, 2, S], BF, tag="qkT")
                v_t = attnp.tile([128, NSB, NHC * HD], BF, tag="v")
                ctxT = attnp.tile([128, NHC, S], BF, tag="ctxT")
                wd_t = attnp.tile([128, NHC, H], BF, tag="wd")
                r1c_t = attnp.tile([CROWS, G, H], BF, tag="r1c")
                rsel_t = attnp.tile([CROWS, 4, 128], BF, tag="rsel")

                # ---- Stage A: hid DMA, LN1, transpose, QKV per panel ----
                with (
                    tc.tile_pool(name="wpoolA", bufs=1) as wpoolA,
                    tc.tile_pool(name="workA", bufs=2) as workA,
                    tc.tile_pool(name="pA_t", bufs=2, space="PSUM") as pA_t,
                    tc.tile_pool(name="pA_qk", bufs=2, space="PSUM") as pA_qk,
                    tc.tile_pool(name="pA_v", bufs=2, space="PSUM") as pA_v,
                ):
                    hbs = {}
                    for sb in range(4):  # panel-0 rows first
                        hb = workA.tile([128, H], BF, tag="hidblk", bufs=6,
                                        name=f"hb{sb}")
                        nc.sync.dma_start(
                            out=hb, in_=hid[sb * 128:(sb + 1) * 128, :])
                        hbs[sb] = hb
                    fstage = wpoolA.tile([128, H], FP, tag="fc2bstage")
                    load_consts(fstage)
                    wqk_t = wpoolA.tile([128, NHCH, 4 * HD], BF, tag="wqk")
                    nc.sync.dma_start(
                        out=wqk_t,
                        in_=wqk[:, :].rearrange("(c p) f -> p c f", p=128))
                    wv_t = wpoolA.tile([128, NHCH, NHC * HD], BF, tag="wv")
                    nc.sync.dma_start(
                        out=wv_t,
                        in_=wv[:, :].rearrange("(c p) f -> p c f", p=128))
                    for sb in range(4, NSB):
                        hb = workA.tile([128, H], BF, tag="hidblk", bufs=6,
                                        name=f"hb{sb}")
                        nc.sync.dma_start(
                            out=hb, in_=hid[sb * 128:(sb + 1) * 128, :])
                        hbs[sb] = hb
                    nc.sync.dma_start(
                        out=wd_t,
                        in_=wd[:, :].rearrange("(c p) f -> p c f", p=128))
                    nc.sync.dma_start(
                        out=r1c_t,
                        in_=res1[:, :].rearrange("(g r) h -> r g h", r=CROWS))
                    nc.sync.dma_start(
                        out=rsel_t,
                        in_=rsel[:, :].rearrange("k (b p) -> k b p", p=128))

                    for p in range(NPANEL):
                        xhat_blocks = []
                        for sb4 in range(PANEL // 128):
                            sb = p * (PANEL // 128) + sb4
                            rstd, nmurs = ln_rowstats(hbs[sb])
                            xh = workA.tile([128, H], BF, tag="xhat", bufs=4)
                            nc.scalar.activation(xh, hbs[sb], AF.Identity,
                                                 bias=nmurs, scale=rstd)
                            xhat_blocks.append(xh)

                        xT = workA.tile([128, NHCH, PANEL], BF, tag="xT")
                        for hc in range(NHCH):
                            pt = pA_t.tile([128, PANEL], BF, tag="mmT")
                            for sb4 in range(PANEL // 128):
                                nc.tensor.transpose(
                                    pt[:, sb4 * 128:(sb4 + 1) * 128],
                                    xhat_blocks[sb4][:, hc * 128:(hc + 1) * 128],
                                    ident)
                            if hc % 2 == 0:
                                nc.vector.tensor_copy(xT[:, hc, :], pt)
                            else:
                                nc.scalar.copy(xT[:, hc, :], pt)

                        for fb in range(4):  # q_h0, k_h0, q_h1, k_h1
                            pq = pA_qk.tile([128, PANEL], FP, tag="mmqk")
                            for hc in range(NHCH):
                                nc.tensor.matmul(
                                    pq, wqk_t[:, hc, fb * 128:(fb + 1) * 128],
                                    xT[:, hc, :],
                                    start=(hc == 0), stop=(hc == NHCH - 1))
                            head, isk = fb // 2, fb % 2
                            nc.scalar.activation(
                                qkT[:, head, isk, p * PANEL:(p + 1) * PANEL],
                                pq, AF.Identity, bias=bqk_t[:, fb:fb + 1],
                                scale=1.0)

                        for sb4 in range(PANEL // 128):
                            blk = p * (PANEL // 128) + sb4
                            pv = pA_v.tile([128, NHC * HD], FP, tag="mmv")
                            for hc in range(NHCH):
                                nc.tensor.matmul(
                                    pv, xT[:, hc, sb4 * 128:(sb4 + 1) * 128],
                                    wv_t[:, hc, :],
                                    start=(hc == 0), stop=(hc == NHCH - 1))
                            nc.vector.tensor_copy(v_t[:, blk, :], pv)

                # ==== Stage C/D/E: attention + dense + chunked RS1/AG ====
                with (
                    tc.tile_pool(name="workC", bufs=2) as workC,
                    tc.tile_pool(name="psc", bufs=2, space="PSUM") as psc,
                    tc.tile_pool(name="psw", bufs=2, space="PSUM") as psw,
                    tc.tile_pool(name="psx", bufs=2, space="PSUM") as psx,
                ):
                    def attention_head(g, head):
                        """scores/softmax/ctx for q-rows [g*512,(g+1)*512)."""
                        probs_g, dn_g = [], []
                        for ib in range(4 * g, 4 * g + 4):
                            jw = (ib + 1) * 128
                            qblk = qkT[:, head, 0, ib * 128:(ib + 1) * 128]
                            nch = (jw + 1023) // 1024
                            pch = []
                            for cc in range(nch):
                                c0 = cc * 1024
                                w_ = min(1024, jw - c0)
                                pp = psc.tile([128, 1024], FP, tag="sc")
                                pch.append((pp, c0, w_))
                            # pass 1: scores (stationary q-block)
                            for pp, c0, w_ in pch:
                                for s0 in range(0, w_, 512):
                                    sw = min(512, w_ - s0)
                                    nc.tensor.matmul(
                                        pp[:, s0:s0 + sw], qblk,
                                        qkT[:, head, 1, c0 + s0:c0 + s0 + sw],
                                        start=True, stop=False,
                                        skip_group_check=True)
                            # pass 2: + alibi (rank-2: coarse+fine rows)
                            for pp, c0, w_ in pch:
                                for s0 in range(0, w_, 512):
                                    sw = min(512, w_ - s0)
                                    j0 = head * S + c0 + s0
                                    isdiag = (c0 + s0 + sw == jw)
                                    nc.tensor.matmul(
                                        pp[:, s0:s0 + sw], ones2,
                                        alibsp_t[:, j0:j0 + sw],
                                        start=False, stop=not isdiag,
                                        skip_group_check=True)
                            # pass 3: causal mask on the diagonal block
                            pp, c0, w_ = pch[-1]
                            d0 = (jw - 128) - c0
                            nc.tensor.matmul(
                                pp[:, d0:d0 + 128], cmT, ident,
                                start=False, stop=True,
                                skip_group_check=True)
                            # exp (PSUM-read) + denominators
                            probs = workC.tile([128, S], BF, tag="probs",
                                               bufs=5)
                            dparts = []
                            for ci, (pp, c0, w_) in enumerate(pch):
                                dp = stats.tile([128, 1], FP, tag=f"dnm{ci}",
                                                name=f"dnm{ci}")
                                nc.scalar.activation(
                                    probs[:, c0:c0 + w_], pp[:, :w_], AF.Exp,
                                    bias=nalib_t[:, head, ib:ib + 1],
                                    scale=1.0, accum_out=dp)
                                dparts.append(dp)
                            if len(dparts) == 2:
                                nc.vector.tensor_add(dparts[0], dparts[0],
                                                     dparts[1])
                            invd = stats.tile([128, 1], FP, tag="invd")
                            nc.vector.reciprocal(invd, dparts[0])
                            dn = workC.tile([128, 128], BF, tag="dn", bufs=5)
                            nc.vector.tensor_scalar_mul(dn, ident, invd)
                            probs_g.append(probs)
                            dn_g.append(dn)

                        pctx = psx.tile([128, 512], FP, tag="acc")
                        njc = 4 * g + 4

                        def wt_fill(jc):
                            """probs.T x diag(1/denom) for key-block jc."""
                            ib0 = max(jc, 4 * g)
                            nblk = 4 * g + 4 - ib0
                            pw = psw.tile([128, 512], FP, tag="wt")
                            for k, ib in enumerate(range(ib0, 4 * g + 4)):
                                nc.tensor.matmul(
                                    pw[:, k * 128:(k + 1) * 128],
                                    probs_g[ib - 4 * g][:, jc * 128:(jc + 1) * 128],
                                    dn_g[ib - 4 * g],
                                    start=True, stop=True)
                            wts = workC.tile([128, 512], BF, tag="wts",
                                             bufs=3)
                            nc.vector.tensor_copy(wts[:, :nblk * 128],
                                                  pw[:, :nblk * 128])
                            return wts, ib0, nblk

                        # software-pipelined: fill pw(jc+1) on the PE while
                        # ctx(jc) waits for its DVE wts copy, instead of a
                        # hard PE->DVE->PE round-trip every iteration.
                        cur = wt_fill(0)
                        for jc in range(njc):
                            nxt = wt_fill(jc + 1) if jc + 1 < njc else None
                            wts, ib0, nblk = cur
                            off = (ib0 - 4 * g) * 128
                            nc.tensor.matmul(
                                pctx[:, off:off + nblk * 128],
                                v_t[:, jc, head * HD:(head + 1) * HD],
                                wts[:, :nblk * 128],
                                start=(jc == 0), stop=(jc == njc - 1),
                                skip_group_check=True)
                            cur = nxt
                        nc.vector.tensor_copy(
                            ctxT[:, head, g * 512:(g + 1) * 512], pctx)

                    def dense_rs1(g):
                        """dense partial for rows [g*512,(g+1)*512) + RS1.

                        A final K=64 one-hot selector matmul adds res1 for
                        exactly the rows this core owns, so rs1_out is the
                        FINISHED attention row and the AllGather can chain
                        directly off the ReduceScatter."""
                        dsb = workC.tile([128, 4, H], BF, tag="densebf")
                        for sb4 in range(4):
                            blk = 4 * g + sb4
                            for fc in range(4):
                                pd = psx.tile([128, 512], FP, tag="acc")
                                for h in range(NHC):
                                    nc.tensor.matmul(
                                        pd,
                                        ctxT[:, h, blk * 128:(blk + 1) * 128],
                                        wd_t[:, h, fc * 512:(fc + 1) * 512],
                                        start=(h == 0), stop=False,
                                        skip_group_check=True)
                                nc.tensor.matmul(
                                    pd, rsel_t[:, sb4, :],
                                    r1c_t[:, g, fc * 512:(fc + 1) * 512],
                                    start=False, stop=True,
                                    skip_group_check=True)
                                if fc % 2 == 0:
                                    nc.vector.tensor_copy(
                                        dsb[:, sb4, fc * 512:(fc + 1) * 512],
                                        pd)
                                else:
                                    nc.scalar.copy(
                                        dsb[:, sb4, fc * 512:(fc + 1) * 512],
                                        pd)
                        nc.sync.dma_start(
                            out=rs1_in[g][:, :].rearrange("(b p) h -> p b h",
                                                          p=128),
                            in_=dsb)
                        nc.gpsimd.collective_compute(
                            "ReduceScatter", ALU.add, replica_groups=rg,
                            ins=[rs1_in[g].opt()], outs=[rs1_out[g].opt()])

                    # stage_e is pure-gpsimd, so it can follow its dense
                    # immediately: AG_g fires at the earliest possible
                    # moment and no compute queue ever blocks on it.
                    for idx, g in enumerate(GORDER):
                        for head in range(NHC):
                            attention_head(g, head)
                        dense_rs1(g)
                        stage_e(g)

            # ======== Stage F: MLP + chunked RS2; Stage G epilogue ========
            with (
                tc.tile_pool(name="mlpw", bufs=1) as mlpw,
                tc.tile_pool(name="workF", bufs=2) as workF,
                tc.tile_pool(name="pf", bufs=2, space="PSUM") as pf_pool,
                tc.tile_pool(name="pt3", bufs=2, space="PSUM") as pt3_pool,
                tc.tile_pool(name="pm", bufs=2, space="PSUM") as pm_pool,
            ):
                # first chunk's gather BEFORE the weights: it unblocks the
                # transposes; wf1's first half follows (FC1 needs it next),
                # then wf2 (needed ~27us later).
                g0 = GORDER[0]
                ytmp0 = workF.tile([128, 4, H], BF, tag="ytmp", bufs=1,
                                   name="ytmp0")
                nc.sync.dma_start(
                    out=ytmp0,
                    in_=ag_out[g0][:, :, :].rearrange(
                        "c r h -> (c r) h").rearrange(
                        "(b p) h -> p b h", p=128))
                wf1_t = mlpw.tile([128, NHCH, F1C], BF, tag="wf1")
                nc.sync.dma_start(
                    out=wf1_t[:, :, :F1C // 2],
                    in_=wf1[:, :F1C // 2].rearrange("(c p) f -> p c f", p=128))
                nc.sync.dma_start(
                    out=wf1_t[:, :, F1C // 2:],
                    in_=wf1[:, F1C // 2:].rearrange("(c p) f -> p c f", p=128))
                wf2_t = mlpw.tile([128, NF1, H], BF, tag="wf2")
                nc.sync.dma_start(
                    out=wf2_t,
                    in_=wf2[:, :].rearrange("(c p) f -> p c f", p=128))

                def stage_g(g):
                    """rows of chunk g: out = rs2 + attn(+fc2b).

                    rs2 read waits on the RS2_g collective -> gpsimd queue."""
                    rsb2 = workF.tile([CROWS, H], BF, tag="rs2blk")
                    nc.gpsimd.dma_start(out=rsb2, in_=rs2_out[g])
                    ob = workF.tile([CROWS, H], FP, tag="outblk", bufs=1)
                    nc.gpsimd.tensor_add(ob, rsb2, attn_c[g])
                    nc.gpsimd.dma_start(out=out[g * CROWS:(g + 1) * CROWS, :],
                                        in_=ob)

                def prep(g, ytmp_pre=None):
                    """gather + replicated LN2 + transpose -> yTs for chunk g.

                    Emitted one chunk ahead (between FC1 and FC2 of the
                    previous chunk) so the DVE/ACT LN work hides under the
                    previous chunk's matmuls."""
                    if ytmp_pre is not None:
                        ytmp = ytmp_pre
                    else:
                        ytmp = workF.tile([128, 4, H], BF, tag="ytmp", bufs=1)
                        nc.gpsimd.dma_start(
                            out=ytmp,
                            in_=ag_out[g][:, :, :].rearrange(
                                "c r h -> (c r) h").rearrange(
                                "(b p) h -> p b h", p=128))
                    for b4 in range(4):
                        rstd, nmurs = ln_rowstats(ytmp[:, b4, :])
                        nc.scalar.activation(ytmp[:, b4, :], ytmp[:, b4, :],
                                             AF.Identity, bias=nmurs,
                                             scale=rstd)
                    yTs = workF.tile([128, NHCH, 512], BF, tag="yTs", bufs=2)
                    for hc in range(NHCH):
                        pt3 = pt3_pool.tile([128, 512], BF, tag="mmT3")
                        for b4 in range(4):
                            nc.tensor.transpose(
                                pt3[:, b4 * 128:(b4 + 1) * 128],
                                ytmp[:, b4, hc * 128:(hc + 1) * 128],
                                ident)
                        if hc % 2 == 0:
                            nc.vector.tensor_copy(yTs[:, hc, :], pt3)
                        else:
                            nc.scalar.copy(yTs[:, hc, :], pt3)
                    return yTs

                yTs_cur = prep(GORDER[0], ytmp_pre=ytmp0)
                for idx, g in enumerate(GORDER):
                    # fetch this core's finished attn rows (residual-2) and
                    # fold fc2b in (overlapped, off the tail)
                    nc.gpsimd.dma_start(out=attn_c[g], in_=rs1_out[g])
                    nc.gpsimd.tensor_add(attn_c[g], attn_c[g],
                                         fc2b_t[0:CROWS, :])
                    # FC1: hdnT[f1, seq-chunk] = gelu(Wf1 @ yT + b)
                    hdnT = workF.tile([128, NF1, 512], BF, tag="hdnT")
                    for f1c in range(NF1):
                        pf = pf_pool.tile([128, 512], FP, tag="mmf1")
                        for hc in range(NHCH):
                            nc.tensor.matmul(
                                pf, wf1_t[:, hc, f1c * 128:(f1c + 1) * 128],
                                yTs_cur[:, hc, :],
                                start=(hc == 0), stop=(hc == NHCH - 1))
                        nc.scalar.activation(
                            hdnT[:, f1c, :], pf,
                            AF.Identity if SIM_MODE else AF.Gelu_apprx_tanh,
                            bias=bf1_t[:, f1c:f1c + 1], scale=1.0)
                    # FC2 partial + RS2_g
                    for sb4 in range(4):
                        msb = workF.tile([128, H], BF, tag="mlpbf", bufs=2)
                        for half in range(2):
                            pm = pm_pool.tile([128, 1024], FP, tag="mmf2")
                            for f1c in range(NF1):
                                st = hdnT[:, f1c, sb4 * 128:(sb4 + 1) * 128]
                                for fc in range(2):
                                    f0 = half * 1024 + fc * 512
                                    nc.tensor.matmul(
                                        pm[:, fc * 512:(fc + 1) * 512],
                                        st, wf2_t[:, f1c, f0:f0 + 512],
                                        start=(f1c == 0),
                                        stop=(f1c == NF1 - 1),
                                        skip_group_check=True)
                            if half == 0:
                                nc.vector.tensor_copy(
                                    msb[:, half * 1024:(half + 1) * 1024], pm)
                            else:
                                nc.scalar.copy(
                                    msb[:, half * 1024:(half + 1) * 1024], pm)
                        nc.sync.dma_start(
                            out=rs2_in[g][sb4 * 128:(sb4 + 1) * 128, :],
                            in_=msb)
                    nc.gpsimd.collective_compute(
                        "ReduceScatter", ALU.add, replica_groups=rg,
                        ins=[rs2_in[g].opt()], outs=[rs2_out[g].opt()])
                    if idx >= 1:
                        stage_g(GORDER[idx - 1])
                    # next chunk's gather/LN2/transpose: AFTER this chunk's
                    # matmuls in the PE stream, so a late AllGather can only
                    # stall work that genuinely needs it.
                    if idx + 1 < len(GORDER):
                        yTs_cur = prep(GORDER[idx + 1])
                stage_g(GORDER[-1])
    nc.compile()
    return nc


def _host_prep(inputs):
    """Slice/fold weights per core. Returns list of per-core input maps."""
    bf16 = ml_dtypes.bfloat16
    hs = np.asarray(inputs["hidden_states"], np.float32).reshape(S, H)
    g1 = np.asarray(inputs["ln1_g"], np.float32)
    b1 = np.asarray(inputs["ln1_b"], np.float32)
    qkv_w = np.asarray(inputs["qkv_w"], np.float32)
    qkv_b = np.asarray(inputs["qkv_b"], np.float32)
    dense_w = np.asarray(inputs["dense_w"], np.float32)
    dense_b = np.asarray(inputs["dense_b"], np.float32)
    g2 = np.asarray(inputs["ln2_g"], np.float32)
    b2 = np.asarray(inputs["ln2_b"], np.float32)
    fc1_w = np.asarray(inputs["fc1_w"], np.float32)
    fc1_b = np.asarray(inputs["fc1_b"], np.float32)
    fc2_w = np.asarray(inputs["fc2_w"], np.float32)
    fc2_b = np.asarray(inputs["fc2_b"], np.float32)
    alibi = np.asarray(inputs["alibi"], np.float32).reshape(NH, S)

    inv = 1.0 / np.sqrt(np.float32(HD))
    hs_bf = hs.astype(bf16)

    # v-bias contribution to the dense output (softmax rows sum to 1):
    # ctx = ctx_nobv + bv  =>  dense += bv_cat @ dense_w.T  (fold into res1)
    bv_cat = np.zeros(H, np.float32)
    for h in range(NH):
        vr = qkv_w[h * 3 * HD + 2 * HD:h * 3 * HD + 3 * HD, :]
        bv_cat[h * HD:(h + 1) * HD] = (
            qkv_b[h * 3 * HD + 2 * HD:h * 3 * HD + 3 * HD] + vr @ b1)
    dense_b_eff = dense_b + bv_cat @ dense_w.T

    in_maps = []
    for c in range(NCORE):
        heads = [NHC * c + i for i in range(NHC)]
        wqk_cols, bqk_rows, wv_cols = [], [], []
        for h in heads:
            qr = qkv_w[h * 3 * HD:h * 3 * HD + HD, :]
            kr = qkv_w[h * 3 * HD + HD:h * 3 * HD + 2 * HD, :]
            vr = qkv_w[h * 3 * HD + 2 * HD:h * 3 * HD + 3 * HD, :]
            qb = qkv_b[h * 3 * HD:h * 3 * HD + HD] + qr @ b1
            kb = qkv_b[h * 3 * HD + HD:h * 3 * HD + 2 * HD] + kr @ b1
            wqk_cols.append((qr * g1[None, :]).T * inv)
            wqk_cols.append((kr * g1[None, :]).T)
            bqk_rows.append(qb * inv)
            bqk_rows.append(kb)
            wv_cols.append((vr * g1[None, :]).T)
        rows_c = np.concatenate(
            [np.arange(g * GROWS + c * CROWS, g * GROWS + (c + 1) * CROWS)
             for g in range(G)])
        # one-hot selector: within a 512-row chunk, this core owns rows
        # [c*64, (c+1)*64) -> block b4=c//2, partitions (c%2)*64 + k
        rsel_np = np.zeros((CROWS, 4, 128), np.float32)
        for k in range(CROWS):
            rsel_np[k, c // 2, (c % 2) * CROWS + k] = 1.0
        rsel_c = np.ascontiguousarray(rsel_np.reshape(CROWS, 512)).astype(bf16)
        alibi_c = alibi[heads[0]:heads[-1] + 1, :]          # [NHC, S]
        # coarse part: multiples of 8 (exact in bf16 up to 2048);
        # fine part in [-4, 4] (bf16 rounding ~2^-8 relative).
        acoarse = (8.0 * np.round(alibi_c / 8.0)).astype(bf16)
        afine = (alibi_c - acoarse.astype(np.float32)).astype(bf16)
        # the exp bias is the exact fp32 negation of the on-device sum
        nalib_c = -(acoarse.astype(np.float32) + afine.astype(np.float32))
        in_maps.append({
            "hid": hs_bf,
            "wqk": np.ascontiguousarray(
                np.concatenate(wqk_cols, axis=1)).astype(bf16),
            "bqk": np.ascontiguousarray(np.stack(bqk_rows, axis=0)),
            "wv": np.ascontiguousarray(
                np.concatenate(wv_cols, axis=1)).astype(bf16),
            "alibsp": np.ascontiguousarray(
                np.stack([acoarse.reshape(-1), afine.reshape(-1)], axis=0)),
            "nalib": np.ascontiguousarray(nalib_c.reshape(NHC, NSB, 128)),
            "wd": np.ascontiguousarray(
                dense_w[:, heads[0] * HD:(heads[-1] + 1) * HD].T).astype(bf16),
            "res1": np.ascontiguousarray(
                hs[rows_c, :] + dense_b_eff[None, :]).astype(bf16),
            "rsel": rsel_c,
            "wf1": np.ascontiguousarray(
                (fc1_w[c * F1C:(c + 1) * F1C, :] * g2[None, :]).T).astype(bf16),
            "bf1": np.ascontiguousarray(
                (fc1_b[c * F1C:(c + 1) * F1C]
                 + fc1_w[c * F1C:(c + 1) * F1C, :] @ b2
                 ).reshape(NF1, 128)),
            "wf2": np.ascontiguousarray(
                fc2_w[:, c * F1C:(c + 1) * F1C].T).astype(bf16),
            "fc2b": fc2_b,
        })
    return in_maps


def _assemble(results) -> np.ndarray:
    """Reassemble the strided row ownership into the full [B, S, H]."""
    full = np.empty((S, H), np.float32)
    for c in range(NCORE):
        shard = np.asarray(results[c]["out"], np.float32)  # [SSH, H]
        for g in range(G):
            full[g * GROWS + c * CROWS:g * GROWS + (c + 1) * CROWS, :] = \
                shard[g * CROWS:(g + 1) * CROWS, :]
    return full.reshape(B, S, H)


_CACHED_NC = None


def kernel(**inputs) -> np.ndarray:
    global _CACHED_NC
    in_maps = _host_prep(inputs)
    if _CACHED_NC is None:
        _CACHED_NC = build_program()
    res = run_bass_kernel_spmd(_CACHED_NC, in_maps, list(range(NCORE)))
    return _assemble(res.results)


# revision 42
# speedup vs baseline: 1.0639x; 1.0042x over previous
"""BloomBlock (B=1, S=2048, H=2048, NH=16) on 8 Trainium2 NeuronCores.

Megatron tensor-parallel: each core owns 2 attention heads and 1024 rows of
the 8192-wide MLP. LN1 (+transpose) is replicated; attention/MLP partial
sums are reduce-scattered in bf16; LN2 runs on the local sequence slices;
normalized activations are all-gathered for the MLP.

v2 (throughput rework):
  * All three collectives are CHUNKED over 4 sequence groups of 512 rows
    and issued as soon as their producer chunk finishes, so they overlap
    attention / MLP compute instead of idling every engine. Row ownership
    becomes strided: core c owns global rows {g*512 + c*64 + r}; the host
    reassembles.
  * Alibi and the causal mask are accumulated INTO the score PSUM by the
    tensor engine (a K=2 rank-2 matmul with ones against a bf16
    coarse+fine alibi split, and a transposed-causal-mask matmul), so no
    vector-engine pass over the [128, jw] scores is needed; exp reads
    PSUM directly.
  * Softmax drops the row-max pass: exp uses bias = -alibi[row] (alibi is
    monotonically increasing along the row, scores are O(5), so the
    exponent is bounded above by ~5 and the diagonal term keeps the
    denominator away from 0). The bias is the exact fp32 negation of the
    coarse+fine sum, so the diagonal exponent is exactly 0.
  * LN stats via bn_stats/bn_aggr (one DVE pass instead of reduce_sum +
    Square-accumulate).
  * DMAs consolidated (one descriptor per weight matrix, 2-D/3-D access
    patterns) - the baseline issued 690 DMAs at ~0.7us Sync issue each.
  * PSUM->SBUF copies widened and balanced across Vector/Scalar engines.
  * hidden_states streamed in bf16 (the fp32 residual arrives via res1);
    the v bias is folded into res1 on the host (softmax rows sum to 1).
"""
import sys

for _p in ("/opt/trn_rl_repo",):
    if _p not in sys.path:
        sys.path.insert(0, _p)

import numpy as np
import ml_dtypes

import concourse.bass as bass
from concourse import bacc
import concourse.mybir as mybir
import concourse.tile as tile
from concourse.bass_utils import run_bass_kernel_spmd
from concourse.masks import make_identity, make_lower_triangular

AF = mybir.ActivationFunctionType
ALU = mybir.AluOpType
AX = mybir.AxisListType

B, S, H, NH, HD = 1, 2048, 2048, 16, 128
NCORE = 8
NHC = NH // NCORE          # heads per core = 2
SSH = S // NCORE           # rows owned per core = 256
F1 = 4 * H                 # 8192
F1C = F1 // NCORE          # 1024
NSB = S // 128             # 16 s-blocks
NHCH = H // 128            # 16 h-chunks
NF1 = F1C // 128           # 8
EPS = 1e-5
MASK_NEG = -1e30
G = 4                      # collective chunks (512 rows each)
GROWS = S // G             # 512
CROWS = GROWS // NCORE     # 64 rows per core per chunk

FP = mybir.dt.float32
BF = mybir.dt.bfloat16

SIM_MODE = False           # CoreSim lacks Gelu; use Identity there
PANEL = 512                # stage-A transpose/QKV panel width
NPANEL = S // PANEL        # 4
GORDER = [0, 1, 2, 3]      # ascending: RS1_0/AG_0 fire early, F starts hole-free


def build_program():
    nc = bacc.Bacc("TRN2", target_bir_lowering=False, debug=False,
                   enable_asserts=False, num_devices=NCORE)

    # ---------------- I/O ----------------
    hid = nc.declare_dram_parameter("hid", [S, H], BF, isOutput=False)
    wqk = nc.declare_dram_parameter("wqk", [H, 4 * HD], BF, isOutput=False)
    bqk = nc.declare_dram_parameter("bqk", [4, HD], FP, isOutput=False)
    wv = nc.declare_dram_parameter("wv", [H, NHC * HD], BF, isOutput=False)
    alibsp = nc.declare_dram_parameter("alibsp", [2, NHC * S], BF,
                                       isOutput=False)
    nalib = nc.declare_dram_parameter("nalib", [NHC, NSB, 128], FP,
                                      isOutput=False)
    wd = nc.declare_dram_parameter("wd", [NHC * HD, H], BF, isOutput=False)
    res1 = nc.declare_dram_parameter("res1", [SSH, H], BF, isOutput=False)
    rsel = nc.declare_dram_parameter("rsel", [CROWS, 4 * 128], BF,
                                     isOutput=False)
    wf1 = nc.declare_dram_parameter("wf1", [H, F1C], BF, isOutput=False)
    bf1 = nc.declare_dram_parameter("bf1", [NF1, 128], FP, isOutput=False)
    wf2 = nc.declare_dram_parameter("wf2", [F1C, H], BF, isOutput=False)
    fc2b = nc.declare_dram_parameter("fc2b", [H], FP, isOutput=False)
    out = nc.declare_dram_parameter("out", [SSH, H], FP, isOutput=True)

    rg = [list(range(NCORE))]

    with tile.TileContext(nc) as tc:
        with (
            tc.tile_pool(name="dram", bufs=1, space="DRAM") as dram,
            tc.tile_pool(name="consts", bufs=1) as consts,
            tc.tile_pool(name="stats", bufs=4) as stats,
            tc.tile_pool(name="postp", bufs=1) as postp,
        ):
            # ------- collective bounce buffers (per chunk) -------
            rs1_in = [dram.tile([GROWS, H], BF, tag=f"rs1i{g}", name=f"rs1i{g}")
                      for g in range(G)]
            rs1_out = [dram.tile([CROWS, H], BF, tag=f"rs1o{g}",
                                 name=f"rs1o{g}") for g in range(G)]
            ag_out = [dram.tile([NCORE, CROWS, H], BF, tag=f"ago{g}",
                                name=f"ago{g}", addr_space="Shared")
                      for g in range(G)]
            rs2_in = [dram.tile([GROWS, H], BF, tag=f"rs2i{g}", name=f"rs2i{g}")
                      for g in range(G)]
            rs2_out = [dram.tile([CROWS, H], BF, tag=f"rs2o{g}",
                                 name=f"rs2o{g}") for g in range(G)]

            # ------------ constants (non-DMA first; DMAs after hid) ------
            ident = consts.tile([128, 128], BF, tag="ident")
            make_identity(nc, ident)
            # transposed causal mask: cmT[a,b] = MASK_NEG iff a > b, so
            # (cmT.T @ I)[i,j] = MASK_NEG iff j > i  (strictly-future).
            cmT = consts.tile([128, 128], BF, tag="cmT")
            make_lower_triangular(nc, cmT, val=MASK_NEG, diag=False)
            ones2 = consts.tile([2, 128], BF, tag="ones2")
            nc.vector.memset(ones2, 1.0)
            eps_t = consts.tile([128, 1], FP, tag="eps")
            nc.vector.memset(eps_t, EPS)
            bqk_t = consts.tile([128, 4], FP, tag="bqk")
            nalib_t = consts.tile([128, NHC, NSB], FP, tag="nalib")
            alibsp_t = consts.tile([2, NHC * S], BF, tag="alibsp")
            bf1_t = consts.tile([128, NF1], FP, tag="bf1")
            fc2b_t = consts.tile([128, H], FP, tag="fc2b")

            def load_consts(fstage):
                nc.sync.dma_start(out=bqk_t,
                                  in_=bqk[:, :].rearrange("b p -> p b"))
                nc.sync.dma_start(
                    out=nalib_t,
                    in_=nalib[:, :, :].rearrange("h b p -> p h b"))
                nc.sync.dma_start(out=alibsp_t, in_=alibsp[:, :])
                nc.sync.dma_start(out=bf1_t,
                                  in_=bf1[:, :].rearrange("b p -> p b"))
                nc.sync.dma_start(out=fstage[0:1, :], in_=fc2b[None, :])
                nc.gpsimd.partition_broadcast(fc2b_t, fstage[0:1, :])

            # fp32 attn rows (LN2 input + residual-2), per chunk; spans C..G
            attn_c = [postp.tile([CROWS, H], BF, tag=f"attn{g}",
                                 name=f"attn{g}") for g in range(G)]
            # stage-E scratch; spans C..F (E of the last groups is emitted
            # inside the stage-F stream)
            epool = postp

            def ln_rowstats(src):
                """bn_stats mean/var over the free axis of src [P, H].

                Returns (rstd, -mu*rstd) as [P, 1] fp32."""
                p = src.shape[0]
                bs = stats.tile([128, H // 512, 6], FP, tag="bnst")
                for c in range(H // 512):
                    nc.vector.bn_stats(bs[:p, c, :],
                                       src[:, c * 512:(c + 1) * 512])
                mv = stats.tile([128, 2], FP, tag="bnmv")
                nc.vector.bn_aggr(
                    mv[:p], bs[:p].rearrange("p c s -> p (c s)"))
                std = stats.tile([128, 1], FP, tag="std")
                nc.scalar.activation(std[:p], mv[:p, 1:2], AF.Sqrt,
                                     bias=eps_t[:p])
                rstd = stats.tile([128, 1], FP, tag="rstd")
                nc.vector.reciprocal(rstd[:p], std[:p])
                nmurs = stats.tile([128, 1], FP, tag="nmurs")
                nc.vector.tensor_mul(nmurs[:p], mv[:p, 0:1], rstd[:p])
                nc.vector.tensor_scalar_mul(nmurs[:p], nmurs[:p], -1.0)
                return rstd[:p], nmurs[:p]

            def stage_e(g):
                """AG of the finished (residual-included) attn rows.

                res1 was folded into the dense partial by the selector
                matmul, so rs1_out IS the attention output: the AllGather
                chains directly off the ReduceScatter on the CC stream
                with no intermediate compute."""
                nc.gpsimd.collective_compute(
                    "AllGather", ALU.bypass, replica_groups=rg,
                    ins=[rs1_out[g].opt()], outs=[ag_out[g].opt()])

            # ==== persistent attention activations (stages A..C) ====
            with tc.tile_pool(name="attnp", bufs=1) as attnp:
                qkT = attnp.tile([128, NHC, 2, S], BF, tag="qkT")
                v_t = attnp.tile([128, NSB, NHC * HD], BF, tag="v")
                ctxT = attnp.tile([128, NHC, S], BF, tag="ctxT")
                wd_t = attnp.tile([128, NHC, H], BF, tag="wd")
                r1c_t = attnp.tile([CROWS, G, H], BF, tag="r1c")
                rsel_t = attnp.tile([CROWS, 4, 128], BF, tag="rsel")

                # ---- Stage A: hid DMA, LN1, transpose, QKV per panel ----
                with (
                    tc.tile_pool(name="wpoolA", bufs=1) as wpoolA,
                    tc.tile_pool(name="workA", bufs=2) as workA,
                    tc.tile_pool(name="pA_t", bufs=2, space="PSUM") as pA_t,
                    tc.tile_pool(name="pA_qk", bufs=2, space="PSUM") as pA_qk,
                    tc.tile_pool(name="pA_v", bufs=2, space="PSUM") as pA_v,
                ):
                    hbs = {}
                    for sb in range(4):  # panel-0 rows first
                        hb = workA.tile([128, H], BF, tag="hidblk", bufs=6,
                                        name=f"hb{sb}")
                        nc.sync.dma_start(
                            out=hb, in_=hid[sb * 128:(sb + 1) * 128, :])
                        hbs[sb] = hb
                    fstage = wpoolA.tile([128, H], FP, tag="fc2bstage")
                    load_consts(fstage)
                    wqk_t = wpoolA.tile([128, NHCH, 4 * HD], BF, tag="wqk")
                    nc.sync.dma_start(
                        out=wqk_t,
                        in_=wqk[:, :].rearrange("(c p) f -> p c f", p=128))
                    wv_t = wpoolA.tile([128, NHCH, NHC * HD], BF, tag="wv")
                    nc.sync.dma_start(
                        out=wv_t,
                        in_=wv[:, :].rearrange("(c p) f -> p c f", p=128))
                    for sb in range(4, NSB):
                        hb = workA.tile([128, H], BF, tag="hidblk", bufs=6,
                                        name=f"hb{sb}")
                        nc.sync.dma_start(
                            out=hb, in_=hid[sb * 128:(sb + 1) * 128, :])
                        hbs[sb] = hb
                    nc.sync.dma_start(
                        out=wd_t,
                        in_=wd[:, :].rearrange("(c p) f -> p c f", p=128))
                    nc.sync.dma_start(
                        out=r1c_t,
                        in_=res1[:, :].rearrange("(g r) h -> r g h", r=CROWS))
                    nc.sync.dma_start(
                        out=rsel_t,
                        in_=rsel[:, :].rearrange("k (b p) -> k b p", p=128))

                    for p in range(NPANEL):
                        xhat_blocks = []
                        for sb4 in range(PANEL // 128):
                            sb = p * (PANEL // 128) + sb4
                            rstd, nmurs = ln_rowstats(hbs[sb])
                            xh = workA.tile([128, H], BF, tag="xhat", bufs=4)
                            nc.scalar.activation(xh, hbs[sb], AF.Identity,
                                                 bias=nmurs, scale=rstd)
                            xhat_blocks.append(xh)

                        xT = workA.tile([128, NHCH, PANEL], BF, tag="xT")
                        for hc in range(NHCH):
                            pt = pA_t.tile([128, PANEL], BF, tag="mmT")
                            for sb4 in range(PANEL // 128):
                                nc.tensor.transpose(
                                    pt[:, sb4 * 128:(sb4 + 1) * 128],
                                    xhat_blocks[sb4][:, hc * 128:(hc + 1) * 128],
                                    ident)
                            if hc % 2 == 0:
                                nc.vector.tensor_copy(xT[:, hc, :], pt)
                            else:
                                nc.scalar.copy(xT[:, hc, :], pt)

                        for fb in range(4):  # q_h0, k_h0, q_h1, k_h1
                            pq = pA_qk.tile([128, PANEL], FP, tag="mmqk")
                            for hc in range(NHCH):
                                nc.tensor.matmul(
                                    pq, wqk_t[:, hc, fb * 128:(fb + 1) * 128],
                                    xT[:, hc, :],
                                    start=(hc == 0), stop=(hc == NHCH - 1))
                            head, isk = fb // 2, fb % 2
                            nc.scalar.activation(
                                qkT[:, head, isk, p * PANEL:(p + 1) * PANEL],
                                pq, AF.Identity, bias=bqk_t[:, fb:fb + 1],
                                scale=1.0)

                        for sb4 in range(PANEL // 128):
                            blk = p * (PANEL // 128) + sb4
                            pv = pA_v.tile([128, NHC * HD], FP, tag="mmv")
                            for hc in range(NHCH):
                                nc.tensor.matmul(
                                    pv, xT[:, hc, sb4 * 128:(sb4 + 1) * 128],
                                    wv_t[:, hc, :],
                                    start=(hc == 0), stop=(hc == NHCH - 1))
                            nc.vector.tensor_copy(v_t[:, blk, :], pv)

                # ==== Stage C/D/E: attention + dense + chunked RS1/AG ====
                with (
                    tc.tile_pool(name="workC", bufs=2) as workC,
                    tc.tile_pool(name="psc", bufs=2, space="PSUM") as psc,
                    tc.tile_pool(name="psw", bufs=2, space="PSUM") as psw,
                    tc.tile_pool(name="psx", bufs=2, space="PSUM") as psx,
                ):
                    def attention_head(g, head):
                        """scores/softmax/ctx for q-rows [g*512,(g+1)*512)."""
                        probs_g, dn_g = [], []
                        for ib in range(4 * g, 4 * g + 4):
                            jw = (ib + 1) * 128
                            qblk = qkT[:, head, 0, ib * 128:(ib + 1) * 128]
                            nch = (jw + 1023) // 1024
                            # per-chunk: scores + alibi (+mask) then exp
                            # IMMEDIATELY, so chunk 0's exp runs on ACT
                            # while the PE fills chunk 1 - the psc ring
                            # slot frees a chunk earlier and the next
                            # q-block's scores never wait on exp latency.
                            probs = workC.tile([128, S], BF, tag="probs",
                                               bufs=5)
                            dparts = []
                            for cc in range(nch):
                                c0 = cc * 1024
                                w_ = min(1024, jw - c0)
                                islast = (cc == nch - 1)
                                pp = psc.tile([128, 1024], FP, tag="sc")
                                for s0 in range(0, w_, 512):
                                    sw = min(512, w_ - s0)
                                    nc.tensor.matmul(
                                        pp[:, s0:s0 + sw], qblk,
                                        qkT[:, head, 1, c0 + s0:c0 + s0 + sw],
                                        start=True, stop=False,
                                        skip_group_check=True)
                                for s0 in range(0, w_, 512):
                                    sw = min(512, w_ - s0)
                                    j0 = head * S + c0 + s0
                                    isdiag = (c0 + s0 + sw == jw)
                                    nc.tensor.matmul(
                                        pp[:, s0:s0 + sw], ones2,
                                        alibsp_t[:, j0:j0 + sw],
                                        start=False, stop=not isdiag,
                                        skip_group_check=True)
                                if islast:
                                    # causal mask on the diagonal block
                                    d0 = (jw - 128) - c0
                                    nc.tensor.matmul(
                                        pp[:, d0:d0 + 128], cmT, ident,
                                        start=False, stop=True,
                                        skip_group_check=True)
                                dp = stats.tile([128, 1], FP,
                                                tag=f"dnm{cc}",
                                                name=f"dnm{cc}")
                                nc.scalar.activation(
                                    probs[:, c0:c0 + w_], pp[:, :w_], AF.Exp,
                                    bias=nalib_t[:, head, ib:ib + 1],
                                    scale=1.0, accum_out=dp)
                                dparts.append(dp)
                            if len(dparts) == 2:
                                nc.vector.tensor_add(dparts[0], dparts[0],
                                                     dparts[1])
                            invd = stats.tile([128, 1], FP, tag="invd")
                            nc.vector.reciprocal(invd, dparts[0])
                            dn = workC.tile([128, 128], BF, tag="dn", bufs=5)
                            nc.vector.tensor_scalar_mul(dn, ident, invd)
                            probs_g.append(probs)
                            dn_g.append(dn)

                        pctx = psx.tile([128, 512], FP, tag="acc")
                        njc = 4 * g + 4

                        def wt_fill(jc):
                            """probs.T x diag(1/denom) for key-block jc."""
                            ib0 = max(jc, 4 * g)
                            nblk = 4 * g + 4 - ib0
                            pw = psw.tile([128, 512], FP, tag="wt")
                            for k, ib in enumerate(range(ib0, 4 * g + 4)):
                                nc.tensor.matmul(
                                    pw[:, k * 128:(k + 1) * 128],
                                    probs_g[ib - 4 * g][:, jc * 128:(jc + 1) * 128],
                                    dn_g[ib - 4 * g],
                                    start=True, stop=True)
                            wts = workC.tile([128, 512], BF, tag="wts",
                                             bufs=3)
                            nc.vector.tensor_copy(wts[:, :nblk * 128],
                                                  pw[:, :nblk * 128])
                            return wts, ib0, nblk

                        # software-pipelined: fill pw(jc+1) on the PE while
                        # ctx(jc) waits for its DVE wts copy, instead of a
                        # hard PE->DVE->PE round-trip every iteration.
                        cur = wt_fill(0)
                        for jc in range(njc):
                            nxt = wt_fill(jc + 1) if jc + 1 < njc else None
                            wts, ib0, nblk = cur
                            off = (ib0 - 4 * g) * 128
                            nc.tensor.matmul(
                                pctx[:, off:off + nblk * 128],
                                v_t[:, jc, head * HD:(head + 1) * HD],
                                wts[:, :nblk * 128],
                                start=(jc == 0), stop=(jc == njc - 1),
                                skip_group_check=True)
                            cur = nxt
                        nc.vector.tensor_copy(
                            ctxT[:, head, g * 512:(g + 1) * 512], pctx)

                    def dense_rs1(g):
                        """dense partial for rows [g*512,(g+1)*512) + RS1.

                        A final K=64 one-hot selector matmul adds res1 for
                        exactly the rows this core owns, so rs1_out is the
                        FINISHED attention row and the AllGather can chain
                        directly off the ReduceScatter."""
                        dsb = workC.tile([128, 4, H], BF, tag="densebf")
                        for sb4 in range(4):
                            blk = 4 * g + sb4
                            for fc in range(4):
                                pd = psx.tile([128, 512], FP, tag="acc")
                                for h in range(NHC):
                                    nc.tensor.matmul(
                                        pd,
                                        ctxT[:, h, blk * 128:(blk + 1) * 128],
                                        wd_t[:, h, fc * 512:(fc + 1) * 512],
                                        start=(h == 0), stop=False,
                                        skip_group_check=True)
                                nc.tensor.matmul(
                                    pd, rsel_t[:, sb4, :],
                                    r1c_t[:, g, fc * 512:(fc + 1) * 512],
                                    start=False, stop=True,
                                    skip_group_check=True)
                                if fc % 2 == 0:
                                    nc.vector.tensor_copy(
                                        dsb[:, sb4, fc * 512:(fc + 1) * 512],
                                        pd)
                                else:
                                    nc.scalar.copy(
                                        dsb[:, sb4, fc * 512:(fc + 1) * 512],
                                        pd)
                        nc.sync.dma_start(
                            out=rs1_in[g][:, :].rearrange("(b p) h -> p b h",
                                                          p=128),
                            in_=dsb)
                        nc.gpsimd.collective_compute(
                            "ReduceScatter", ALU.add, replica_groups=rg,
                            ins=[rs1_in[g].opt()], outs=[rs1_out[g].opt()])

                    # stage_e is pure-gpsimd, so it can follow its dense
                    # immediately: AG_g fires at the earliest possible
                    # moment and no compute queue ever blocks on it.
                    for idx, g in enumerate(GORDER):
                        for head in range(NHC):
                            attention_head(g, head)
                        dense_rs1(g)
                        stage_e(g)

            # ======== Stage F: MLP + chunked RS2; Stage G epilogue ========
            with (
                tc.tile_pool(name="mlpw", bufs=1) as mlpw,
                tc.tile_pool(name="workF", bufs=2) as workF,
                tc.tile_pool(name="pf", bufs=2, space="PSUM") as pf_pool,
                tc.tile_pool(name="pt3", bufs=2, space="PSUM") as pt3_pool,
                tc.tile_pool(name="pm", bufs=2, space="PSUM") as pm_pool,
            ):
                # first chunk's gather BEFORE the weights: it unblocks the
                # transposes; wf1's first half follows (FC1 needs it next),
                # then wf2 (needed ~27us later).
                g0 = GORDER[0]
                ytmp0 = workF.tile([128, 4, H], BF, tag="ytmp", bufs=1,
                                   name="ytmp0")
                nc.sync.dma_start(
                    out=ytmp0,
                    in_=ag_out[g0][:, :, :].rearrange(
                        "c r h -> (c r) h").rearrange(
                        "(b p) h -> p b h", p=128))
                wf1_t = mlpw.tile([128, NHCH, F1C], BF, tag="wf1")
                nc.sync.dma_start(
                    out=wf1_t[:, :, :F1C // 2],
                    in_=wf1[:, :F1C // 2].rearrange("(c p) f -> p c f", p=128))
                nc.sync.dma_start(
                    out=wf1_t[:, :, F1C // 2:],
                    in_=wf1[:, F1C // 2:].rearrange("(c p) f -> p c f", p=128))
                wf2_t = mlpw.tile([128, NF1, H], BF, tag="wf2")
                nc.sync.dma_start(
                    out=wf2_t,
                    in_=wf2[:, :].rearrange("(c p) f -> p c f", p=128))

                def stage_g(g):
                    """rows of chunk g: out = rs2 + attn(+fc2b).

                    rs2 read waits on the RS2_g collective -> gpsimd queue."""
                    rsb2 = workF.tile([CROWS, H], BF, tag="rs2blk")
                    nc.gpsimd.dma_start(out=rsb2, in_=rs2_out[g])
                    ob = workF.tile([CROWS, H], FP, tag="outblk", bufs=1)
                    nc.gpsimd.tensor_add(ob, rsb2, attn_c[g])
                    nc.gpsimd.dma_start(out=out[g * CROWS:(g + 1) * CROWS, :],
                                        in_=ob)

                def prep(g, ytmp_pre=None):
                    """gather + replicated LN2 + transpose -> yTs for chunk g.

                    Emitted one chunk ahead (between FC1 and FC2 of the
                    previous chunk) so the DVE/ACT LN work hides under the
                    previous chunk's matmuls."""
                    if ytmp_pre is not None:
                        ytmp = ytmp_pre
                    else:
                        ytmp = workF.tile([128, 4, H], BF, tag="ytmp", bufs=1)
                        nc.gpsimd.dma_start(
                            out=ytmp,
                            in_=ag_out[g][:, :, :].rearrange(
                                "c r h -> (c r) h").rearrange(
                                "(b p) h -> p b h", p=128))
                    for b4 in range(4):
                        rstd, nmurs = ln_rowstats(ytmp[:, b4, :])
                        nc.scalar.activation(ytmp[:, b4, :], ytmp[:, b4, :],
                                             AF.Identity, bias=nmurs,
                                             scale=rstd)
                    yTs = workF.tile([128, NHCH, 512], BF, tag="yTs", bufs=2)
                    for hc in range(NHCH):
                        pt3 = pt3_pool.tile([128, 512], BF, tag="mmT3")
                        for b4 in range(4):
                            nc.tensor.transpose(
                                pt3[:, b4 * 128:(b4 + 1) * 128],
                                ytmp[:, b4, hc * 128:(hc + 1) * 128],
                                ident)
                        if hc % 2 == 0:
                            nc.vector.tensor_copy(yTs[:, hc, :], pt3)
                        else:
                            nc.scalar.copy(yTs[:, hc, :], pt3)
                    return yTs

                yTs_cur = prep(GORDER[0], ytmp_pre=ytmp0)
                for idx, g in enumerate(GORDER):
                    # fetch this core's finished attn rows (residual-2) and
                    # fold fc2b in (overlapped, off the tail)
                    nc.gpsimd.dma_start(out=attn_c[g], in_=rs1_out[g])
                    nc.gpsimd.tensor_add(attn_c[g], attn_c[g],
                                         fc2b_t[0:CROWS, :])
                    # FC1: hdnT[f1, seq-chunk] = gelu(Wf1 @ yT + b)
                    hdnT = workF.tile([128, NF1, 512], BF, tag="hdnT")
                    for f1c in range(NF1):
                        pf = pf_pool.tile([128, 512], FP, tag="mmf1")
                        for hc in range(NHCH):
                            nc.tensor.matmul(
                                pf, wf1_t[:, hc, f1c * 128:(f1c + 1) * 128],
                                yTs_cur[:, hc, :],
                                start=(hc == 0), stop=(hc == NHCH - 1))
                        nc.scalar.activation(
                            hdnT[:, f1c, :], pf,
                            AF.Identity if SIM_MODE else AF.Gelu_apprx_tanh,
                            bias=bf1_t[:, f1c:f1c + 1], scale=1.0)
                    # FC2 partial + RS2_g
                    for sb4 in range(4):
                        msb = workF.tile([128, H], BF, tag="mlpbf", bufs=2)
                        for half in range(2):
                            pm = pm_pool.tile([128, 1024], FP, tag="mmf2")
                            for f1c in range(NF1):
                                st = hdnT[:, f1c, sb4 * 128:(sb4 + 1) * 128]
                                for fc in range(2):
                                    f0 = half * 1024 + fc * 512
                                    nc.tensor.matmul(
                                        pm[:, fc * 512:(fc + 1) * 512],
                                        st, wf2_t[:, f1c, f0:f0 + 512],
                                        start=(f1c == 0),
                                        stop=(f1c == NF1 - 1),
                                        skip_group_check=True)
                            if half == 0:
                                nc.vector.tensor_copy(
                                    msb[:, half * 1024:(half + 1) * 1024], pm)
                            else:
                                nc.scalar.copy(
                                    msb[:, half * 1024:(half + 1) * 1024], pm)
                        nc.sync.dma_start(
                            out=rs2_in[g][sb4 * 128:(sb4 + 1) * 128, :],
                            in_=msb)
                    nc.gpsimd.collective_compute(
                        "ReduceScatter", ALU.add, replica_groups=rg,
                        ins=[rs2_in[g].opt()], outs=[rs2_out[g].opt()])
                    if idx >= 1:
                        stage_g(GORDER[idx - 1])
                    # next chunk's gather/LN2/transpose: AFTER this chunk's
                    # matmuls in the PE stream, so a late AllGather can only
                    # stall work that genuinely needs it.
                    if idx + 1 < len(GORDER):
                        yTs_cur = prep(GORDER[idx + 1])
                stage_g(GORDER[-1])
    nc.compile()
    return nc


def _host_prep(inputs):
    """Slice/fold weights per core. Returns list of per-core input maps."""
    bf16 = ml_dtypes.bfloat16
    hs = np.asarray(inputs["hidden_states"], np.float32).reshape(S, H)
    g1 = np.asarray(inputs["ln1_g"], np.float32)
    b1 = np.asarray(inputs["ln1_b"], np.float32)
    qkv_w = np.asarray(inputs["qkv_w"], np.float32)
    qkv_b = np.asarray(inputs["qkv_b"], np.float32)
    dense_w = np.asarray(inputs["dense_w"], np.float32)
    dense_b = np.asarray(inputs["dense_b"], np.float32)
    g2 = np.asarray(inputs["ln2_g"], np.float32)
    b2 = np.asarray(inputs["ln2_b"], np.float32)
    fc1_w = np.asarray(inputs["fc1_w"], np.float32)
    fc1_b = np.asarray(inputs["fc1_b"], np.float32)
    fc2_w = np.asarray(inputs["fc2_w"], np.float32)
    fc2_b = np.asarray(inputs["fc2_b"], np.float32)
    alibi = np.asarray(inputs["alibi"], np.float32).reshape(NH, S)

    inv = 1.0 / np.sqrt(np.float32(HD))
    hs_bf = hs.astype(bf16)

    # v-bias contribution to the dense output (softmax rows sum to 1):
    # ctx = ctx_nobv + bv  =>  dense += bv_cat @ dense_w.T  (fold into res1)
    bv_cat = np.zeros(H, np.float32)
    for h in range(NH):
        vr = qkv_w[h * 3 * HD + 2 * HD:h * 3 * HD + 3 * HD, :]
        bv_cat[h * HD:(h + 1) * HD] = (
            qkv_b[h * 3 * HD + 2 * HD:h * 3 * HD + 3 * HD] + vr @ b1)
    dense_b_eff = dense_b + bv_cat @ dense_w.T

    in_maps = []
    for c in range(NCORE):
        heads = [NHC * c + i for i in range(NHC)]
        wqk_cols, bqk_rows, wv_cols = [], [], []
        for h in heads:
            qr = qkv_w[h * 3 * HD:h * 3 * HD + HD, :]
            kr = qkv_w[h * 3 * HD + HD:h * 3 * HD + 2 * HD, :]
            vr = qkv_w[h * 3 * HD + 2 * HD:h * 3 * HD + 3 * HD, :]
            qb = qkv_b[h * 3 * HD:h * 3 * HD + HD] + qr @ b1
            kb = qkv_b[h * 3 * HD + HD:h * 3 * HD + 2 * HD] + kr @ b1
            wqk_cols.append((qr * g1[None, :]).T * inv)
            wqk_cols.append((kr * g1[None, :]).T)
            bqk_rows.append(qb * inv)
            bqk_rows.append(kb)
            wv_cols.append((vr * g1[None, :]).T)
        rows_c = np.concatenate(
            [np.arange(g * GROWS + c * CROWS, g * GROWS + (c + 1) * CROWS)
             for g in range(G)])
        # one-hot selector: within a 512-row chunk, this core owns rows
        # [c*64, (c+1)*64) -> block b4=c//2, partitions (c%2)*64 + k
        rsel_np = np.zeros((CROWS, 4, 128), np.float32)
        for k in range(CROWS):
            rsel_np[k, c // 2, (c % 2) * CROWS + k] = 1.0
        rsel_c = np.ascontiguousarray(rsel_np.reshape(CROWS, 512)).astype(bf16)
        alibi_c = alibi[heads[0]:heads[-1] + 1, :]          # [NHC, S]
        # coarse part: multiples of 8 (exact in bf16 up to 2048);
        # fine part in [-4, 4] (bf16 rounding ~2^-8 relative).
        acoarse = (8.0 * np.round(alibi_c / 8.0)).astype(bf16)
        afine = (alibi_c - acoarse.astype(np.float32)).astype(bf16)
        # the exp bias is the exact fp32 negation of the on-device sum
        nalib_c = -(acoarse.astype(np.float32) + afine.astype(np.float32))
        in_maps.append({
            "hid": hs_bf,
            "wqk": np.ascontiguousarray(
                np.concatenate(wqk_cols, axis=1)).astype(bf16),
            "bqk": np.ascontiguousarray(np.stack(bqk_rows, axis=0)),
            "wv": np.ascontiguousarray(
                np.concatenate(wv_cols, axis=1)).astype(bf16),
            "alibsp": np.ascontiguousarray(
                np.stack([acoarse.reshape(-1), afine.reshape(-1)], axis=0)),
            "nalib": np.ascontiguousarray(nalib_c.reshape(NHC, NSB, 128)),
            "wd": np.ascontiguousarray(
                dense_w[:, heads[0] * HD:(heads[-1] + 1) * HD].T).astype(bf16),
            "res1": np.ascontiguousarray(
                hs[rows_c, :] + dense_b_eff[None, :]).astype(bf16),
            "rsel": rsel_c,
            "wf1": np.ascontiguousarray(
                (fc1_w[c * F1C:(c + 1) * F1C, :] * g2[None, :]).T).astype(bf16),
            "bf1": np.ascontiguousarray(
                (fc1_b[c * F1C:(c + 1) * F1C]
                 + fc1_w[c * F1C:(c + 1) * F1C, :] @ b2
                 ).reshape(NF1, 128)),
            "wf2": np.ascontiguousarray(
                fc2_w[:, c * F1C:(c + 1) * F1C].T).astype(bf16),
            "fc2b": fc2_b,
        })
    return in_maps


def _assemble(results) -> np.ndarray:
    """Reassemble the strided row ownership into the full [B, S, H]."""
    full = np.empty((S, H), np.float32)
    for c in range(NCORE):
        shard = np.asarray(results[c]["out"], np.float32)  # [SSH, H]
        for g in range(G):
            full[g * GROWS + c * CROWS:g * GROWS + (c + 1) * CROWS, :] = \
                shard[g * CROWS:(g + 1) * CROWS, :]
    return full.reshape(B, S, H)


_CACHED_NC = None


def kernel(**inputs) -> np.ndarray:
    global _CACHED_NC
    in_maps = _host_prep(inputs)
    if _CACHED_NC is None:
        _CACHED_NC = build_program()
    res = run_bass_kernel_spmd(_CACHED_NC, in_maps, list(range(NCORE)))
    return _assemble(res.results)
